# revision 2
# baseline (speedup 1.0000x reference)
"""Trainium2 Bass kernel for nn_AttentionSACModel (sparse_attention).

Data-parallel across 8 NeuronCores: obs sharded along batch, params replicated.
On-device layout keeps batch on the matmul free dim (activations stored
feature-major / transposed); all host<->device layout changes happen in numpy.
"""
import sys
import os

if "/opt/trn_rl_repo" not in sys.path:
    sys.path.insert(0, "/opt/trn_rl_repo")

import numpy as np
import ml_dtypes
_bf16np = ml_dtypes.bfloat16

OWN_DIM = 7
INT_DIM = 7
N_INTR = 20
H = 3
D = 42
TOT = H * D            # 126
ATTN = 128
HID = 256
NOUT = 4
B = 32768
N_CORES = 8
BC = B // N_CORES      # 4096 rows per core
NB = 512               # batch tile (matmul free dim)
NT = BC // NB          # 8 tiles per core
ALPHA = 0.2            # leaky relu slope

_BUILT = {}


def _build_nc():
    import concourse.bacc as bacc
    import concourse.tile as tile
    from concourse import mybir

    f32 = mybir.dt.float32
    f32r = mybir.dt.float32r
    bf16 = mybir.dt.bfloat16
    AF = mybir.ActivationFunctionType
    ALU = mybir.AluOpType
    AX = mybir.AxisListType

    nc = bacc.Bacc()

    # ---- DRAM I/O ----
    xo_d = nc.dram_tensor("xo", [OWN_DIM, BC], f32r, kind="ExternalInput")
    xa_d = nc.dram_tensor("xa", [126, BC], f32r, kind="ExternalInput")       # interactors 0..17, row 7n+f
    xb_d = nc.dram_tensor("xb", [14, BC], f32r, kind="ExternalInput")        # interactors 18,19
    wia_d = nc.dram_tensor("wia", [126, 18 * 126], f32r, kind="ExternalInput")  # padded int-embed lhsT, n<18
    wib_d = nc.dram_tensor("wib", [14, 2 * 126], f32r, kind="ExternalInput")    # n=18,19
    wo_d = nc.dram_tensor("wo", [7, 126], f32r, kind="ExternalInput")
    wq_d = nc.dram_tensor("wqb", [126, 126], f32r, kind="ExternalInput")
    wk_d = nc.dram_tensor("wkb", [126, 126], f32r, kind="ExternalInput")
    wv_d = nc.dram_tensor("wvb", [126, 126], f32r, kind="ExternalInput")
    va_d = nc.dram_tensor("va32", [126, 32], bf16, kind="ExternalInput")
    ds_d = nc.dram_tensor("densel", [128, 3], f32r, kind="ExternalInput")
    eb_d = nc.dram_tensor("ebcsel", [128, 4 * 126], f32r, kind="ExternalInput")
    rb_d = nc.dram_tensor("rbc", [3, 126], f32r, kind="ExternalInput")
    wat_d = nc.dram_tensor("wat", [126, 128], f32r, kind="ExternalInput")
    wop_d = nc.dram_tensor("wop", [126, 128], f32r, kind="ExternalInput")
    wh1_d = nc.dram_tensor("wh1r", [128, 512], f32r, kind="ExternalInput")   # [p, kc*256+m]
    wh2_d = nc.dram_tensor("wh2r", [128, 512], f32r, kind="ExternalInput")
    wout_d = nc.dram_tensor("woutr", [128, 8], f32r, kind="ExternalInput")   # [p, kc*4+m]
    bown_d = nc.dram_tensor("bown", [126, 1], f32, kind="ExternalInput")
    bint_d = nc.dram_tensor("bint", [126, 1], f32, kind="ExternalInput")
    bat_d = nc.dram_tensor("bat", [128, 1], f32, kind="ExternalInput")
    bop_d = nc.dram_tensor("bop", [128, 1], f32, kind="ExternalInput")
    bh1_d = nc.dram_tensor("bh1", [128, 2], f32, kind="ExternalInput")
    bh2_d = nc.dram_tensor("bh2", [128, 2], f32, kind="ExternalInput")
    bout_d = nc.dram_tensor("bout", [4, 1], f32, kind="ExternalInput")
    out_d = nc.dram_tensor("outT", [NOUT, BC], f32, kind="ExternalOutput")

    with tile.TileContext(nc) as tc:
        with tc.tile_pool(name="const", bufs=1) as cst, \
             tc.tile_pool(name="px", bufs=2) as px, \
             tc.tile_pool(name="pemb", bufs=3) as pemb, \
             tc.tile_pool(name="peng", bufs=3) as peng, \
             tc.tile_pool(name="pE", bufs=7) as pE, \
             tc.tile_pool(name="pv", bufs=2) as pv, \
             tc.tile_pool(name="pp", bufs=1) as pp, \
             tc.tile_pool(name="ph", bufs=2) as ph, \
             tc.tile_pool(name="mega", bufs=2, space="PSUM") as mega, \
             tc.tile_pool(name="small", bufs=4, space="PSUM") as small:

            # ---- load constants ----
            WiA = cst.tile([126, 18 * 126], f32r)
            WiB = cst.tile([14, 2 * 126], f32r)
            Wo = cst.tile([7, 126], f32r)
            Wq = cst.tile([126, 126], f32r)
            Wk = cst.tile([126, 126], f32r)
            Wv = cst.tile([126, 126], f32r)
            Va = cst.tile([126, 32], bf16)
            Ds = cst.tile([128, 3], f32r)
            Eb = cst.tile([128, 4 * 126], f32r)
            Rb = cst.tile([3, 126], f32r)
            Wat = cst.tile([126, 128], f32r)
            Wop = cst.tile([126, 128], f32r)
            WH1 = cst.tile([128, 512], f32r)
            WH2 = cst.tile([128, 512], f32r)
            WOUT = cst.tile([128, 8], f32r)
            Bown = cst.tile([126, 1], f32)
            Bint = cst.tile([126, 1], f32)
            Bat = cst.tile([128, 1], f32)
            Bop = cst.tile([128, 1], f32)
            BH1 = cst.tile([128, 2], f32)
            BH2 = cst.tile([128, 2], f32)
            Bout = cst.tile([4, 1], f32)
            for t_sb, t_dr in [(WiA, wia_d), (WiB, wib_d), (Wo, wo_d), (Wq, wq_d),
                               (Wk, wk_d), (Wv, wv_d), (Va, va_d), (Ds, ds_d),
                               (Eb, eb_d), (Rb, rb_d), (Wat, wat_d), (Wop, wop_d),
                               (WH1, wh1_d), (WH2, wh2_d), (WOUT, wout_d),
                               (Bown, bown_d), (Bint, bint_d), (Bat, bat_d),
                               (Bop, bop_d), (BH1, bh1_d), (BH2, bh2_d),
                               (Bout, bout_d)]:
                nc.sync.dma_start(out=t_sb, in_=t_dr[:, :])

            with nc.allow_low_precision(reason="bf16/f32r intermediates; final accums are f32"):
                for t in range(NT):
                    bs = t * NB
                    # ---- inputs ----
                    XO = px.tile([OWN_DIM, NB], f32r, tag="xo")
                    XA = px.tile([126, NB], f32r, tag="xa")
                    XB = px.tile([14, NB], f32r, tag="xb")
                    nc.sync.dma_start(out=XO, in_=xo_d[:, bs:bs + NB])
                    nc.sync.dma_start(out=XA, in_=xa_d[:, bs:bs + NB])
                    nc.sync.dma_start(out=XB, in_=xb_d[:, bs:bs + NB])

                    # ---- own embed ----
                    PO = small.tile([128, NB], f32, tag="sm", name="PO")
                    nc.tensor.matmul(PO[0:126, :], Wo, XO)
                    OWN = ph.tile([126, NB], f32r, tag="own", name="OWN")
                    nc.scalar.activation(OWN, PO[0:126, :], AF.Prelu, bias=Bown, alpha=ALPHA)

                    # ---- interactor embed + attention streams (pairs) ----
                    EGs = []
                    VA = pv.tile([126, N_INTR, NB], bf16, tag="va", name="VA")
                    PS = None
                    for p in range(N_INTR // 2):
                        n0, n1 = 2 * p, 2 * p + 1
                        PZ = mega.tile([126, 2 * NB], f32, tag="mg", name="PZ")
                        for i, n in enumerate((n0, n1)):
                            sl = slice(i * NB, (i + 1) * NB)
                            if n < 18:
                                nc.tensor.matmul(PZ[:, sl], WiA[:, n * 126:(n + 1) * 126], XA)
                            else:
                                nc.tensor.matmul(PZ[:, sl], WiB[:, (n - 18) * 126:(n - 17) * 126], XB)
                        ZT = pemb.tile([126, 2 * NB], f32r, tag="zt", name="ZT")
                        nc.scalar.activation(ZT, PZ, AF.Prelu, bias=Bint, alpha=ALPHA)

                        PK = mega.tile([126, 2 * NB], f32, tag="mg", name="PK")
                        for i in range(2):
                            sl = slice(i * NB, (i + 1) * NB)
                            nc.tensor.matmul(PK[:, sl], Wk, ZT[:, sl], start=True, stop=False)
                            nc.tensor.matmul(PK[:, sl], Wq, OWN, start=False, stop=True)
                        EN = peng.tile([126, 2 * NB], bf16, tag="en", name="EN")
                        nc.scalar.activation(EN, PK, AF.Tanh)

                        PV = mega.tile([126, 2 * NB], f32, tag="mg", name="PV")
                        for i in range(2):
                            sl = slice(i * NB, (i + 1) * NB)
                            nc.tensor.matmul(PV[:, sl], Wv, ZT[:, sl])
                        nc.vector.tensor_copy(out=VA[:, n0:n1 + 1, :], in_=PV)

                        # scores for n0,n1 into the group score bank (4 n per bank)
                        if p % 2 == 0:
                            PS = small.tile([128, NB], f32, tag="sm", name="PS")
                        for i, n in enumerate((n0, n1)):
                            j = n % 4
                            sl = slice(i * NB, (i + 1) * NB)
                            nc.tensor.matmul(PS[32 * j:32 * (j + 1), :], Va, EN[:, sl],
                                             tile_position=(0, 32 * j))
                        if p % 2 == 1:
                            EG = pE.tile([128, NB], f32r, tag="eg", name="EG")
                            nc.scalar.activation(EG, PS, AF.Exp)
                            EGs.append(EG)

                    # ---- softmax denominator ----
                    PD = small.tile([128, NB], f32, tag="sm", name="PD")
                    for g in range(5):
                        nc.tensor.matmul(PD[0:3, :], Ds, EGs[g],
                                         start=(g == 0), stop=(g == 4))
                    RD = ph.tile([3, NB], f32r, tag="rd", name="RD")
                    nc.vector.reciprocal(RD, PD[0:3, :])
                    PR = small.tile([128, NB], f32, tag="sm", name="PR")
                    nc.tensor.matmul(PR[0:126, :], Rb, RD)

                    # ---- ctx = (sum_n exp(s_n) * v_n) * (1/den) ----
                    PST = pp.tile([126, NB, N_INTR], bf16, tag="pst", name="PST")
                    for n in range(N_INTR):
                        g, j = n // 4, n % 4
                        PEb = small.tile([128, NB], f32, tag="sm", name="PEb")
                        nc.tensor.matmul(PEb[0:126, :], Eb[:, j * 126:(j + 1) * 126], EGs[g])
                        nc.vector.tensor_tensor(out=PST[:, :, n], in0=PEb[0:126, :],
                                                in1=VA[:, n, :], op=ALU.mult)
                    CTXU = ph.tile([126, NB], f32, tag="ctxu", name="CTXU")
                    nc.vector.tensor_reduce(CTXU, PST[:, :, :], axis=AX.X, op=ALU.add)
                    CTX = ph.tile([126, NB], f32r, tag="ctx", name="CTX")
                    nc.vector.tensor_tensor(out=CTX, in0=CTXU, in1=PR[0:126, :], op=ALU.mult)

                    # ---- head MLP ----
                    PH1 = small.tile([128, NB], f32, tag="sm", name="PH1")
                    nc.tensor.matmul(PH1, Wat, CTX)
                    ATT = ph.tile([128, NB], f32r, tag="att", name="ATT")
                    nc.scalar.activation(ATT, PH1, AF.Tanh, bias=Bat)

                    PH2 = small.tile([128, NB], f32, tag="sm", name="PH2")
                    nc.tensor.matmul(PH2, Wop, OWN)
                    OWV = ph.tile([128, NB], f32r, tag="owv", name="OWV")
                    nc.scalar.activation(OWV, PH2, AF.Tanh, bias=Bop)

                    H1 = []
                    for mh in range(2):
                        PHh = small.tile([128, NB], f32, tag="sm", name="PHh")
                        nc.tensor.matmul(PHh, WH1[:, mh * 128:(mh + 1) * 128], OWV,
                                         start=True, stop=False)
                        nc.tensor.matmul(PHh, WH1[:, 256 + mh * 128:256 + (mh + 1) * 128], ATT,
                                         start=False, stop=True)
                        H1A = ph.tile([128, NB], f32r, tag=f"h1a{mh}", name="H1A")
                        nc.scalar.activation(H1A, PHh, AF.Prelu, bias=BH1[:, mh:mh + 1], alpha=ALPHA)
                        H1.append(H1A)
                    H2 = []
                    for mh in range(2):
                        PHh2 = small.tile([128, NB], f32, tag="sm", name="PHh2")
                        nc.tensor.matmul(PHh2, WH2[:, mh * 128:(mh + 1) * 128], H1[0],
                                         start=True, stop=False)
                        nc.tensor.matmul(PHh2, WH2[:, 256 + mh * 128:256 + (mh + 1) * 128], H1[1],
                                         start=False, stop=True)
                        H2A = ph.tile([128, NB], f32r, tag=f"h2a{mh}", name="H2A")
                        nc.scalar.activation(H2A, PHh2, AF.Prelu, bias=BH2[:, mh:mh + 1], alpha=ALPHA)
                        H2.append(H2A)

                    PO4 = small.tile([128, NB], f32, tag="sm", name="PO4")
                    nc.tensor.matmul(PO4[0:4, :], WOUT[:, 0:4], H2[0], start=True, stop=False)
                    nc.tensor.matmul(PO4[0:4, :], WOUT[:, 4:8], H2[1], start=False, stop=True)
                    OT = ph.tile([4, NB], f32, tag="ot", name="OT")
                    nc.scalar.activation(OT, PO4[0:4, :], AF.Identity, bias=Bout)
                    nc.sync.dma_start(out=out_d[:, bs:bs + NB], in_=OT)

    nc.compile()
    return nc


def _host_prep(inputs):
    """Build per-core input maps (numpy only)."""
    obs = np.ascontiguousarray(inputs["obs"], dtype=np.float32)
    w_own = np.asarray(inputs["w_own"], np.float32)
    w_int = np.asarray(inputs["w_int"], np.float32)
    wq = np.asarray(inputs["wq"], np.float32)
    wk = np.asarray(inputs["wk"], np.float32)
    wv = np.asarray(inputs["wv"], np.float32)
    v_att = np.asarray(inputs["v_att"], np.float32)
    w_attn = np.asarray(inputs["w_attn"], np.float32)
    w_ownp = np.asarray(inputs["w_ownp"], np.float32)
    w_h1 = np.asarray(inputs["w_h1"], np.float32)
    w_h2 = np.asarray(inputs["w_h2"], np.float32)
    w_out = np.asarray(inputs["w_out"], np.float32)

    def blockdiag(w):  # [H, D, D] -> [126, 126]
        out = np.zeros((TOT, TOT), np.float32)
        for h in range(H):
            out[h * D:(h + 1) * D, h * D:(h + 1) * D] = w[h]
        return out

    wia = np.zeros((126, 18 * 126), np.float32)
    for n in range(18):
        wia[7 * n:7 * n + 7, n * 126:(n + 1) * 126] = w_int
    wib = np.zeros((14, 2 * 126), np.float32)
    for n in range(2):
        wib[7 * n:7 * n + 7, n * 126:(n + 1) * 126] = w_int

    va32 = np.zeros((126, 32), np.float32)
    for h in range(H):
        va32[h * D:(h + 1) * D, h] = v_att[h]

    densel = np.zeros((128, 3), np.float32)
    for j in range(4):
        for h in range(H):
            densel[32 * j + h, h] = 1.0

    ebcsel = np.zeros((128, 4 * 126), np.float32)
    for j in range(4):
        for h in range(H):
            ebcsel[32 * j + h, j * 126 + h * D:(j * 126) + (h + 1) * D] = 1.0

    rbc = np.zeros((3, 126), np.float32)
    for h in range(H):
        rbc[h, h * D:(h + 1) * D] = 1.0

    wh1r = np.ascontiguousarray(
        w_h1.reshape(2, 128, HID).transpose(1, 0, 2).reshape(128, 512))
    wh2r = np.ascontiguousarray(
        w_h2.reshape(2, 128, HID).transpose(1, 0, 2).reshape(128, 512))
    woutr = np.ascontiguousarray(
        w_out.reshape(2, 128, NOUT).transpose(1, 0, 2).reshape(128, 8))

    params = {
        "wia": wia, "wib": wib, "wo": w_own,
        "wqb": blockdiag(wq), "wkb": blockdiag(wk), "wvb": blockdiag(wv),
        "va32": va32.astype(_bf16np), "densel": densel, "ebcsel": ebcsel, "rbc": rbc,
        "wat": w_attn, "wop": w_ownp,
        "wh1r": wh1r, "wh2r": wh2r, "woutr": woutr,
        "bown": np.asarray(inputs["b_own"], np.float32).reshape(126, 1),
        "bint": np.asarray(inputs["b_int"], np.float32).reshape(126, 1),
        "bat": np.asarray(inputs["b_attn"], np.float32).reshape(128, 1),
        "bop": np.asarray(inputs["b_ownp"], np.float32).reshape(128, 1),
        "bh1": np.ascontiguousarray(
            np.asarray(inputs["b_h1"], np.float32).reshape(2, 128).T),
        "bh2": np.ascontiguousarray(
            np.asarray(inputs["b_h2"], np.float32).reshape(2, 128).T),
        "bout": np.asarray(inputs["b_out"], np.float32).reshape(4, 1),
    }

    in_maps = []
    for c in range(N_CORES):
        sl = obs[c * BC:(c + 1) * BC]                       # [BC, 147]
        xo = np.ascontiguousarray(sl[:, :OWN_DIM].T)        # [7, BC]
        intr = sl[:, OWN_DIM:].reshape(BC, N_INTR, INT_DIM)  # [BC, 20, 7]
        intrT = intr.transpose(1, 2, 0)                     # [20, 7, BC]
        xa = np.ascontiguousarray(intrT[:18].reshape(126, BC))
        xb = np.ascontiguousarray(intrT[18:].reshape(14, BC))
        m = {"xo": xo, "xa": xa, "xb": xb}
        m.update(params)
        in_maps.append(m)
    return in_maps


def _get_nc():
    if "nc" not in _BUILT:
        _BUILT["nc"] = _build_nc()
    return _BUILT["nc"]


def run(inputs, trace=False):
    from concourse.bass_utils import run_bass_kernel_spmd
    nc = _get_nc()
    in_maps = _host_prep(inputs)
    res = run_bass_kernel_spmd(nc, in_maps, core_ids=list(range(N_CORES)),
                               trace=trace)
    outs = [res.results[c]["outT"] for c in range(N_CORES)]   # each [4, BC]
    full = np.concatenate(outs, axis=1).T                     # [B, 4]
    return np.ascontiguousarray(full, dtype=np.float32), res


def kernel(**inputs):
    out, _ = run(inputs, trace=False)
    return out


# revision 3
# speedup vs baseline: 1.3012x; 1.3012x over previous
"""Trainium2 Bass kernel for nn_AttentionSACModel (sparse_attention).

Data-parallel across 8 NeuronCores: obs sharded along batch, params replicated.
On-device layout keeps batch on the matmul free dim (activations stored
feature-major / transposed); all host<->device layout changes happen in numpy.
"""
import sys
import os

if "/opt/trn_rl_repo" not in sys.path:
    sys.path.insert(0, "/opt/trn_rl_repo")

import numpy as np
import ml_dtypes
_bf16np = ml_dtypes.bfloat16

OWN_DIM = 7
INT_DIM = 7
N_INTR = 20
H = 3
D = 42
TOT = H * D            # 126
ATTN = 128
HID = 256
NOUT = 4
B = 32768
N_CORES = 8
BC = B // N_CORES      # 4096 rows per core
NB = 512               # batch tile (matmul free dim)
NT = BC // NB          # 8 tiles per core
ALPHA = 0.2            # leaky relu slope

_BUILT = {}


def _build_nc():
    import concourse.bacc as bacc
    import concourse.tile as tile
    from concourse import mybir

    f32 = mybir.dt.float32
    f32r = mybir.dt.float32r
    bf16 = mybir.dt.bfloat16
    AF = mybir.ActivationFunctionType
    ALU = mybir.AluOpType
    AX = mybir.AxisListType

    nc = bacc.Bacc()

    # ---- DRAM I/O ----
    xo_d = nc.dram_tensor("xo", [OWN_DIM, BC], f32r, kind="ExternalInput")
    xa_d = nc.dram_tensor("xa", [126, BC], f32r, kind="ExternalInput")       # interactors 0..17, row 7n+f
    xb_d = nc.dram_tensor("xb", [14, BC], f32r, kind="ExternalInput")        # interactors 18,19
    wia_d = nc.dram_tensor("wia", [126, 18 * 126], f32r, kind="ExternalInput")  # padded int-embed lhsT, n<18
    wib_d = nc.dram_tensor("wib", [14, 2 * 126], f32r, kind="ExternalInput")    # n=18,19
    wo_d = nc.dram_tensor("wo", [7, 126], f32r, kind="ExternalInput")
    wq_d = nc.dram_tensor("wqb", [126, 126], f32r, kind="ExternalInput")
    wk_d = nc.dram_tensor("wkb", [126, 126], f32r, kind="ExternalInput")
    wv_d = nc.dram_tensor("wvb", [126, 126], f32r, kind="ExternalInput")
    va_d = nc.dram_tensor("va32", [126, 32], bf16, kind="ExternalInput")
    ds_d = nc.dram_tensor("densel", [128, 3], f32r, kind="ExternalInput")
    eb_d = nc.dram_tensor("ebcsel", [128, 4 * 126], f32r, kind="ExternalInput")
    rb_d = nc.dram_tensor("rbc", [3, 126], f32r, kind="ExternalInput")
    wat_d = nc.dram_tensor("wat", [126, 128], f32r, kind="ExternalInput")
    wop_d = nc.dram_tensor("wop", [126, 128], f32r, kind="ExternalInput")
    wh1_d = nc.dram_tensor("wh1r", [128, 512], f32r, kind="ExternalInput")   # [p, kc*256+m]
    wh2_d = nc.dram_tensor("wh2r", [128, 512], f32r, kind="ExternalInput")
    wout_d = nc.dram_tensor("woutr", [128, 8], f32r, kind="ExternalInput")   # [p, kc*4+m]
    bown_d = nc.dram_tensor("bown", [126, 1], f32, kind="ExternalInput")
    bint_d = nc.dram_tensor("bint", [126, 1], f32, kind="ExternalInput")
    bat_d = nc.dram_tensor("bat", [128, 1], f32, kind="ExternalInput")
    bop_d = nc.dram_tensor("bop", [128, 1], f32, kind="ExternalInput")
    bh1_d = nc.dram_tensor("bh1", [128, 2], f32, kind="ExternalInput")
    bh2_d = nc.dram_tensor("bh2", [128, 2], f32, kind="ExternalInput")
    bout_d = nc.dram_tensor("bout", [4, 1], f32, kind="ExternalInput")
    out_d = nc.dram_tensor("outT", [NOUT, BC], f32, kind="ExternalOutput")

    with tile.TileContext(nc) as tc:
        with tc.tile_pool(name="const", bufs=1) as cst, \
             tc.tile_pool(name="px", bufs=2) as px, \
             tc.tile_pool(name="pemb", bufs=3) as pemb, \
             tc.tile_pool(name="peng", bufs=3) as peng, \
             tc.tile_pool(name="pE", bufs=8) as pE, \
             tc.tile_pool(name="pv", bufs=2) as pv, \
             tc.tile_pool(name="pp", bufs=2) as pp, \
             tc.tile_pool(name="pn", bufs=6) as pn, \
             tc.tile_pool(name="ph", bufs=2) as ph, \
             tc.tile_pool(name="mega", bufs=2, space="PSUM") as mega, \
             tc.tile_pool(name="small", bufs=4, space="PSUM") as small:

            # ---- load constants ----
            WiA = cst.tile([126, 18 * 126], f32r)
            WiB = cst.tile([14, 2 * 126], f32r)
            Wo = cst.tile([7, 126], f32r)
            Wq = cst.tile([126, 126], f32r)
            Wk = cst.tile([126, 126], f32r)
            Wv = cst.tile([126, 126], f32r)
            Va = cst.tile([126, 32], bf16)
            Ds = cst.tile([128, 3], f32r)
            Eb = cst.tile([128, 4 * 126], f32r)
            Rb = cst.tile([3, 126], f32r)
            Wat = cst.tile([126, 128], f32r)
            Wop = cst.tile([126, 128], f32r)
            WH1 = cst.tile([128, 512], f32r)
            WH2 = cst.tile([128, 512], f32r)
            WOUT = cst.tile([128, 8], f32r)
            Bown = cst.tile([126, 1], f32)
            Bint = cst.tile([126, 1], f32)
            Bat = cst.tile([128, 1], f32)
            Bop = cst.tile([128, 1], f32)
            BH1 = cst.tile([128, 2], f32)
            BH2 = cst.tile([128, 2], f32)
            Bout = cst.tile([4, 1], f32)
            for t_sb, t_dr in [(WiA, wia_d), (WiB, wib_d), (Wo, wo_d), (Wq, wq_d),
                               (Wk, wk_d), (Wv, wv_d), (Va, va_d), (Ds, ds_d),
                               (Eb, eb_d), (Rb, rb_d), (Wat, wat_d), (Wop, wop_d),
                               (WH1, wh1_d), (WH2, wh2_d), (WOUT, wout_d),
                               (Bown, bown_d), (Bint, bint_d), (Bat, bat_d),
                               (Bop, bop_d), (BH1, bh1_d), (BH2, bh2_d),
                               (Bout, bout_d)]:
                nc.sync.dma_start(out=t_sb, in_=t_dr[:, :])

            with nc.allow_low_precision(reason="bf16/f32r intermediates; final accums are f32"):
                for t in range(NT):
                    bs = t * NB
                    # ---- inputs ----
                    XO = px.tile([OWN_DIM, NB], f32r, tag="xo")
                    XA = px.tile([126, NB], f32r, tag="xa")
                    XB = px.tile([14, NB], f32r, tag="xb")
                    nc.sync.dma_start(out=XO, in_=xo_d[:, bs:bs + NB])
                    nc.sync.dma_start(out=XA, in_=xa_d[:, bs:bs + NB])
                    nc.sync.dma_start(out=XB, in_=xb_d[:, bs:bs + NB])

                    # ---- own embed ----
                    PO = small.tile([128, NB], f32, tag="sm", name="PO")
                    nc.tensor.matmul(PO[0:126, :], Wo, XO)
                    OWN = ph.tile([126, NB], f32r, tag="own", name="OWN")
                    nc.scalar.activation(OWN, PO[0:126, :], AF.Prelu, bias=Bown, alpha=ALPHA)

                    # ---- interactor embed + attention streams (pairs) ----
                    EGs = []
                    VA = pv.tile([126, N_INTR, NB], bf16, tag="va", name="VA")
                    PS = None
                    for p in range(N_INTR // 2):
                        n0, n1 = 2 * p, 2 * p + 1
                        PZ = mega.tile([126, 2 * NB], f32, tag="mg", name="PZ")
                        for i, n in enumerate((n0, n1)):
                            sl = slice(i * NB, (i + 1) * NB)
                            if n < 18:
                                nc.tensor.matmul(PZ[:, sl], WiA[:, n * 126:(n + 1) * 126], XA)
                            else:
                                nc.tensor.matmul(PZ[:, sl], WiB[:, (n - 18) * 126:(n - 17) * 126], XB)
                        ZT = pemb.tile([126, 2 * NB], f32r, tag="zt", name="ZT")
                        nc.scalar.activation(ZT, PZ, AF.Prelu, bias=Bint, alpha=ALPHA)

                        PK = mega.tile([126, 2 * NB], f32, tag="mg", name="PK")
                        for i in range(2):
                            sl = slice(i * NB, (i + 1) * NB)
                            nc.tensor.matmul(PK[:, sl], Wk, ZT[:, sl], start=True, stop=False)
                            nc.tensor.matmul(PK[:, sl], Wq, OWN, start=False, stop=True)
                        EN = peng.tile([126, 2 * NB], bf16, tag="en", name="EN")
                        nc.scalar.activation(EN, PK, AF.Tanh)

                        PV = mega.tile([126, 2 * NB], f32, tag="mg", name="PV")
                        for i in range(2):
                            sl = slice(i * NB, (i + 1) * NB)
                            nc.tensor.matmul(PV[:, sl], Wv, ZT[:, sl])
                        nc.scalar.activation(VA[:, n0:n1 + 1, :], PV, AF.Copy)

                        # scores for n0,n1 into the group score bank (4 n per bank)
                        if p % 2 == 0:
                            PS = small.tile([128, NB], f32, tag="sm", name="PS")
                        for i, n in enumerate((n0, n1)):
                            j = n % 4
                            sl = slice(i * NB, (i + 1) * NB)
                            nc.tensor.matmul(PS[32 * j:32 * (j + 1), :], Va, EN[:, sl],
                                             tile_position=(0, 32 * j))
                        if p % 2 == 1:
                            EG = pE.tile([128, NB], f32r, tag="eg", name="EG")
                            nc.scalar.activation(EG, PS, AF.Exp)
                            EGs.append(EG)

                    # ---- softmax denominator ----
                    PD = small.tile([128, NB], f32, tag="sm", name="PD")
                    for g in range(5):
                        nc.tensor.matmul(PD[0:3, :], Ds, EGs[g],
                                         start=(g == 0), stop=(g == 4))
                    RD = ph.tile([3, NB], f32r, tag="rd", name="RD")
                    nc.vector.reciprocal(RD, PD[0:3, :])
                    PR = small.tile([128, NB], f32, tag="sm", name="PR")
                    nc.tensor.matmul(PR[0:126, :], Rb, RD)

                    # ---- ctx = (sum_n exp(s_n) * v_n) * (1/den) ----
                    TST = pp.tile([126, NB, N_INTR // 2], bf16, tag="tst", name="TST")
                    PNs = []
                    for n in range(N_INTR):
                        g, j = n // 4, n % 4
                        PEb = small.tile([128, NB], f32, tag="sm", name="PEb")
                        nc.tensor.matmul(PEb[0:126, :], Eb[:, j * 126:(j + 1) * 126], EGs[g])
                        PN = pn.tile([126, NB], bf16, tag="pn", name="PN")
                        nc.vector.tensor_tensor(out=PN, in0=PEb[0:126, :],
                                                in1=VA[:, n, :], op=ALU.mult)
                        PNs.append(PN)
                        if n % 2 == 1:
                            nc.gpsimd.tensor_add(out=TST[:, :, n // 2],
                                                 in0=PNs[n - 1], in1=PNs[n])
                    CTXU = ph.tile([126, NB], f32, tag="ctxu", name="CTXU")
                    nc.vector.tensor_reduce(CTXU, TST[:, :, :], axis=AX.X, op=ALU.add)
                    CTX = ph.tile([126, NB], f32r, tag="ctx", name="CTX")
                    nc.vector.tensor_tensor(out=CTX, in0=CTXU, in1=PR[0:126, :], op=ALU.mult)

                    # ---- head MLP ----
                    PH1 = small.tile([128, NB], f32, tag="sm", name="PH1")
                    nc.tensor.matmul(PH1, Wat, CTX)
                    ATT = ph.tile([128, NB], f32r, tag="att", name="ATT")
                    nc.scalar.activation(ATT, PH1, AF.Tanh, bias=Bat)

                    PH2 = small.tile([128, NB], f32, tag="sm", name="PH2")
                    nc.tensor.matmul(PH2, Wop, OWN)
                    OWV = ph.tile([128, NB], f32r, tag="owv", name="OWV")
                    nc.scalar.activation(OWV, PH2, AF.Tanh, bias=Bop)

                    H1 = []
                    for mh in range(2):
                        PHh = small.tile([128, NB], f32, tag="sm", name="PHh")
                        nc.tensor.matmul(PHh, WH1[:, mh * 128:(mh + 1) * 128], OWV,
                                         start=True, stop=False)
                        nc.tensor.matmul(PHh, WH1[:, 256 + mh * 128:256 + (mh + 1) * 128], ATT,
                                         start=False, stop=True)
                        H1A = ph.tile([128, NB], f32r, tag=f"h1a{mh}", name="H1A")
                        nc.scalar.activation(H1A, PHh, AF.Prelu, bias=BH1[:, mh:mh + 1], alpha=ALPHA)
                        H1.append(H1A)
                    H2 = []
                    for mh in range(2):
                        PHh2 = small.tile([128, NB], f32, tag="sm", name="PHh2")
                        nc.tensor.matmul(PHh2, WH2[:, mh * 128:(mh + 1) * 128], H1[0],
                                         start=True, stop=False)
                        nc.tensor.matmul(PHh2, WH2[:, 256 + mh * 128:256 + (mh + 1) * 128], H1[1],
                                         start=False, stop=True)
                        H2A = ph.tile([128, NB], f32r, tag=f"h2a{mh}", name="H2A")
                        nc.scalar.activation(H2A, PHh2, AF.Prelu, bias=BH2[:, mh:mh + 1], alpha=ALPHA)
                        H2.append(H2A)

                    PO4 = small.tile([128, NB], f32, tag="sm", name="PO4")
                    nc.tensor.matmul(PO4[0:4, :], WOUT[:, 0:4], H2[0], start=True, stop=False)
                    nc.tensor.matmul(PO4[0:4, :], WOUT[:, 4:8], H2[1], start=False, stop=True)
                    OT = ph.tile([4, NB], f32, tag="ot", name="OT")
                    nc.scalar.activation(OT, PO4[0:4, :], AF.Identity, bias=Bout)
                    nc.sync.dma_start(out=out_d[:, bs:bs + NB], in_=OT)

    nc.compile()
    return nc


def _host_prep(inputs):
    """Build per-core input maps (numpy only)."""
    obs = np.ascontiguousarray(inputs["obs"], dtype=np.float32)
    w_own = np.asarray(inputs["w_own"], np.float32)
    w_int = np.asarray(inputs["w_int"], np.float32)
    wq = np.asarray(inputs["wq"], np.float32)
    wk = np.asarray(inputs["wk"], np.float32)
    wv = np.asarray(inputs["wv"], np.float32)
    v_att = np.asarray(inputs["v_att"], np.float32)
    w_attn = np.asarray(inputs["w_attn"], np.float32)
    w_ownp = np.asarray(inputs["w_ownp"], np.float32)
    w_h1 = np.asarray(inputs["w_h1"], np.float32)
    w_h2 = np.asarray(inputs["w_h2"], np.float32)
    w_out = np.asarray(inputs["w_out"], np.float32)

    def blockdiag(w):  # [H, D, D] -> [126, 126]
        out = np.zeros((TOT, TOT), np.float32)
        for h in range(H):
            out[h * D:(h + 1) * D, h * D:(h + 1) * D] = w[h]
        return out

    wia = np.zeros((126, 18 * 126), np.float32)
    for n in range(18):
        wia[7 * n:7 * n + 7, n * 126:(n + 1) * 126] = w_int
    wib = np.zeros((14, 2 * 126), np.float32)
    for n in range(2):
        wib[7 * n:7 * n + 7, n * 126:(n + 1) * 126] = w_int

    va32 = np.zeros((126, 32), np.float32)
    for h in range(H):
        va32[h * D:(h + 1) * D, h] = v_att[h]

    densel = np.zeros((128, 3), np.float32)
    for j in range(4):
        for h in range(H):
            densel[32 * j + h, h] = 1.0

    ebcsel = np.zeros((128, 4 * 126), np.float32)
    for j in range(4):
        for h in range(H):
            ebcsel[32 * j + h, j * 126 + h * D:(j * 126) + (h + 1) * D] = 1.0

    rbc = np.zeros((3, 126), np.float32)
    for h in range(H):
        rbc[h, h * D:(h + 1) * D] = 1.0

    wh1r = np.ascontiguousarray(
        w_h1.reshape(2, 128, HID).transpose(1, 0, 2).reshape(128, 512))
    wh2r = np.ascontiguousarray(
        w_h2.reshape(2, 128, HID).transpose(1, 0, 2).reshape(128, 512))
    woutr = np.ascontiguousarray(
        w_out.reshape(2, 128, NOUT).transpose(1, 0, 2).reshape(128, 8))

    params = {
        "wia": wia, "wib": wib, "wo": w_own,
        "wqb": blockdiag(wq), "wkb": blockdiag(wk), "wvb": blockdiag(wv),
        "va32": va32.astype(_bf16np), "densel": densel, "ebcsel": ebcsel, "rbc": rbc,
        "wat": w_attn, "wop": w_ownp,
        "wh1r": wh1r, "wh2r": wh2r, "woutr": woutr,
        "bown": np.asarray(inputs["b_own"], np.float32).reshape(126, 1),
        "bint": np.asarray(inputs["b_int"], np.float32).reshape(126, 1),
        "bat": np.asarray(inputs["b_attn"], np.float32).reshape(128, 1),
        "bop": np.asarray(inputs["b_ownp"], np.float32).reshape(128, 1),
        "bh1": np.ascontiguousarray(
            np.asarray(inputs["b_h1"], np.float32).reshape(2, 128).T),
        "bh2": np.ascontiguousarray(
            np.asarray(inputs["b_h2"], np.float32).reshape(2, 128).T),
        "bout": np.asarray(inputs["b_out"], np.float32).reshape(4, 1),
    }

    in_maps = []
    for c in range(N_CORES):
        sl = obs[c * BC:(c + 1) * BC]                       # [BC, 147]
        xo = np.ascontiguousarray(sl[:, :OWN_DIM].T)        # [7, BC]
        intr = sl[:, OWN_DIM:].reshape(BC, N_INTR, INT_DIM)  # [BC, 20, 7]
        intrT = intr.transpose(1, 2, 0)                     # [20, 7, BC]
        xa = np.ascontiguousarray(intrT[:18].reshape(126, BC))
        xb = np.ascontiguousarray(intrT[18:].reshape(14, BC))
        m = {"xo": xo, "xa": xa, "xb": xb}
        m.update(params)
        in_maps.append(m)
    return in_maps


def _get_nc():
    if "nc" not in _BUILT:
        _BUILT["nc"] = _build_nc()
    return _BUILT["nc"]


def run(inputs, trace=False):
    from concourse.bass_utils import run_bass_kernel_spmd
    nc = _get_nc()
    in_maps = _host_prep(inputs)
    res = run_bass_kernel_spmd(nc, in_maps, core_ids=list(range(N_CORES)),
                               trace=trace)
    outs = [res.results[c]["outT"] for c in range(N_CORES)]   # each [4, BC]
    full = np.concatenate(outs, axis=1).T                     # [B, 4]
    return np.ascontiguousarray(full, dtype=np.float32), res


def kernel(**inputs):
    out, _ = run(inputs, trace=False)
    return out


# revision 4
# speedup vs baseline: 1.4459x; 1.1112x over previous
"""Trainium2 Bass kernel for nn_AttentionSACModel (sparse_attention).

Data-parallel across 8 NeuronCores: obs sharded along batch, params replicated.
On-device layout keeps batch on the matmul free dim (activations stored
feature-major / transposed); all host<->device layout changes happen in numpy.
"""
import sys
import os

if "/opt/trn_rl_repo" not in sys.path:
    sys.path.insert(0, "/opt/trn_rl_repo")

import numpy as np
import ml_dtypes
_bf16np = ml_dtypes.bfloat16

OWN_DIM = 7
INT_DIM = 7
N_INTR = 20
H = 3
D = 42
TOT = H * D            # 126
ATTN = 128
HID = 256
NOUT = 4
B = 32768
N_CORES = 8
BC = B // N_CORES      # 4096 rows per core
NB = 512               # batch tile (matmul free dim)
NT = BC // NB          # 8 tiles per core
ALPHA = 0.2            # leaky relu slope

_BUILT = {}


def _build_nc():
    import concourse.bacc as bacc
    import concourse.tile as tile
    from concourse import mybir

    f32 = mybir.dt.float32
    f32r = mybir.dt.float32r
    bf16 = mybir.dt.bfloat16
    AF = mybir.ActivationFunctionType
    ALU = mybir.AluOpType
    AX = mybir.AxisListType

    nc = bacc.Bacc()

    # ---- DRAM I/O ----
    xo_d = nc.dram_tensor("xo", [OWN_DIM, BC], f32r, kind="ExternalInput")
    xa_d = nc.dram_tensor("xa", [126, BC], f32r, kind="ExternalInput")       # interactors 0..17, row 7n+f
    xb_d = nc.dram_tensor("xb", [14, BC], f32r, kind="ExternalInput")        # interactors 18,19
    wia_d = nc.dram_tensor("wia", [126, 18 * 126], f32r, kind="ExternalInput")  # padded int-embed lhsT, n<18
    wib_d = nc.dram_tensor("wib", [14, 2 * 126], f32r, kind="ExternalInput")    # n=18,19
    wo_d = nc.dram_tensor("wo", [7, 126], f32r, kind="ExternalInput")
    wq_d = nc.dram_tensor("wqb", [126, 126], f32r, kind="ExternalInput")
    wk_d = nc.dram_tensor("wkb", [126, 126], f32r, kind="ExternalInput")
    wv_d = nc.dram_tensor("wvb", [126, 126], f32r, kind="ExternalInput")
    va_d = nc.dram_tensor("va32", [126, 32], bf16, kind="ExternalInput")
    ds_d = nc.dram_tensor("densel", [128, 3], f32r, kind="ExternalInput")
    eb_d = nc.dram_tensor("ebcsel", [128, 4 * 126], f32r, kind="ExternalInput")
    rb_d = nc.dram_tensor("rbc", [3, 126], f32r, kind="ExternalInput")
    wat_d = nc.dram_tensor("wat", [126, 128], f32r, kind="ExternalInput")
    wop_d = nc.dram_tensor("wop", [126, 128], f32r, kind="ExternalInput")
    wh1_d = nc.dram_tensor("wh1r", [128, 512], f32r, kind="ExternalInput")   # [p, kc*256+m]
    wh2_d = nc.dram_tensor("wh2r", [128, 512], f32r, kind="ExternalInput")
    wout_d = nc.dram_tensor("woutr", [128, 8], f32r, kind="ExternalInput")   # [p, kc*4+m]
    bown_d = nc.dram_tensor("bown", [126, 1], f32, kind="ExternalInput")
    bint_d = nc.dram_tensor("bint", [126, 1], f32, kind="ExternalInput")
    bat_d = nc.dram_tensor("bat", [128, 1], f32, kind="ExternalInput")
    bop_d = nc.dram_tensor("bop", [128, 1], f32, kind="ExternalInput")
    bh1_d = nc.dram_tensor("bh1", [128, 2], f32, kind="ExternalInput")
    bh2_d = nc.dram_tensor("bh2", [128, 2], f32, kind="ExternalInput")
    bout_d = nc.dram_tensor("bout", [4, 1], f32, kind="ExternalInput")
    out_d = nc.dram_tensor("outT", [NOUT, BC], f32, kind="ExternalOutput")

    with tile.TileContext(nc) as tc:
        with tc.tile_pool(name="const", bufs=1) as cst, \
             tc.tile_pool(name="px", bufs=2) as px, \
             tc.tile_pool(name="pemb", bufs=3) as pemb, \
             tc.tile_pool(name="peng", bufs=3) as peng, \
             tc.tile_pool(name="pE", bufs=8) as pE, \
             tc.tile_pool(name="pv", bufs=2) as pv, \
             tc.tile_pool(name="pp", bufs=2) as pp, \
             tc.tile_pool(name="pn", bufs=6) as pn, \
             tc.tile_pool(name="ph", bufs=2) as ph, \
             tc.tile_pool(name="pz", bufs=2, space="PSUM") as ppz, \
             tc.tile_pool(name="pk", bufs=2, space="PSUM") as ppk, \
             tc.tile_pool(name="sm", bufs=3, space="PSUM") as small, \
             tc.tile_pool(name="pd", bufs=1, space="PSUM") as ppd:

            # ---- load constants ----
            WiA = cst.tile([126, 18 * 126], f32r)
            WiB = cst.tile([14, 2 * 126], f32r)
            Wo = cst.tile([7, 126], f32r)
            Wq = cst.tile([126, 126], f32r)
            Wk = cst.tile([126, 126], f32r)
            Wv = cst.tile([126, 126], f32r)
            Va = cst.tile([126, 32], bf16)
            Ds = cst.tile([128, 3], f32r)
            Eb = cst.tile([128, 4 * 126], f32r)
            Rb = cst.tile([3, 126], f32r)
            Wat = cst.tile([126, 128], f32r)
            Wop = cst.tile([126, 128], f32r)
            WH1 = cst.tile([128, 512], f32r)
            WH2 = cst.tile([128, 512], f32r)
            WOUT = cst.tile([128, 8], f32r)
            Bown = cst.tile([126, 1], f32)
            Bint = cst.tile([126, 1], f32)
            Bat = cst.tile([128, 1], f32)
            Bop = cst.tile([128, 1], f32)
            BH1 = cst.tile([128, 2], f32)
            BH2 = cst.tile([128, 2], f32)
            Bout = cst.tile([4, 1], f32)
            for t_sb, t_dr in [(WiA, wia_d), (WiB, wib_d), (Wo, wo_d), (Wq, wq_d),
                               (Wk, wk_d), (Wv, wv_d), (Va, va_d), (Ds, ds_d),
                               (Eb, eb_d), (Rb, rb_d), (Wat, wat_d), (Wop, wop_d),
                               (WH1, wh1_d), (WH2, wh2_d), (WOUT, wout_d),
                               (Bown, bown_d), (Bint, bint_d), (Bat, bat_d),
                               (Bop, bop_d), (BH1, bh1_d), (BH2, bh2_d),
                               (Bout, bout_d)]:
                nc.sync.dma_start(out=t_sb, in_=t_dr[:, :])

            with nc.allow_low_precision(reason="bf16/f32r intermediates; final accums are f32"):
                for t in range(NT):
                    bs = t * NB
                    # ---- inputs ----
                    XO = px.tile([OWN_DIM, NB], f32r, tag="xo")
                    XA = px.tile([126, NB], f32r, tag="xa")
                    XB = px.tile([14, NB], f32r, tag="xb")
                    nc.sync.dma_start(out=XO, in_=xo_d[:, bs:bs + NB])
                    nc.sync.dma_start(out=XA, in_=xa_d[:, bs:bs + NB])
                    nc.sync.dma_start(out=XB, in_=xb_d[:, bs:bs + NB])

                    # ---- own embed ----
                    PO = small.tile([128, NB], f32, tag="sm", name="PO")
                    nc.tensor.matmul(PO[0:126, :], Wo, XO)
                    OWN = ph.tile([126, NB], f32r, tag="own", name="OWN")
                    nc.scalar.activation(OWN, PO[0:126, :], AF.Prelu, bias=Bown, alpha=ALPHA)

                    # ---- interactor embed + attention streams (pairs) ----
                    EGs = []
                    VA = pv.tile([126, N_INTR, NB], bf16, tag="va", name="VA")
                    PS = None
                    for n in range(N_INTR):
                        PZ = ppz.tile([126, NB], f32, tag="pz", name="PZ")
                        if n < 18:
                            nc.tensor.matmul(PZ, WiA[:, n * 126:(n + 1) * 126], XA)
                        else:
                            nc.tensor.matmul(PZ, WiB[:, (n - 18) * 126:(n - 17) * 126], XB)
                        ZT = pemb.tile([126, NB], f32r, tag="zt", name="ZT")
                        nc.scalar.activation(ZT, PZ, AF.Prelu, bias=Bint, alpha=ALPHA)

                        PK = ppk.tile([126, NB], f32, tag="pk", name="PK")
                        nc.tensor.matmul(PK, Wk, ZT, start=True, stop=False)
                        nc.tensor.matmul(PK, Wq, OWN, start=False, stop=True)
                        EN = peng.tile([126, NB], bf16, tag="en", name="EN")
                        nc.scalar.activation(EN, PK, AF.Tanh)

                        PV = small.tile([128, NB], f32, tag="sm", name="PV")
                        nc.tensor.matmul(PV[0:126, :], Wv, ZT)
                        nc.scalar.activation(VA[:, n, :], PV[0:126, :], AF.Copy)

                        j = n % 4
                        if j == 0:
                            PS = small.tile([128, NB], f32, tag="sm", name="PS")
                        nc.tensor.matmul(PS[32 * j:32 * (j + 1), :], Va, EN,
                                         tile_position=(0, 32 * j))
                        if j == 3:
                            EG = pE.tile([128, NB], f32r, tag="eg", name="EG")
                            nc.scalar.activation(EG, PS, AF.Exp)
                            EGs.append(EG)

                    # ---- softmax denominator ----
                    PD = ppd.tile([128, NB], f32, tag="pd", name="PD")
                    for g in range(5):
                        nc.tensor.matmul(PD[0:3, :], Ds, EGs[g],
                                         start=(g == 0), stop=(g == 4))
                    RD = ph.tile([3, NB], f32r, tag="rd", name="RD")
                    nc.vector.reciprocal(RD, PD[0:3, :])
                    PR = small.tile([128, NB], f32, tag="sm", name="PR")
                    nc.tensor.matmul(PR[0:126, :], Rb, RD)

                    # ---- ctx = (sum_n exp(s_n) * v_n) * (1/den) ----
                    TST = pp.tile([126, NB, N_INTR // 2], bf16, tag="tst", name="TST")
                    PNs = []
                    for n in range(N_INTR):
                        g, j = n // 4, n % 4
                        PEb = small.tile([128, NB], f32, tag="sm", name="PEb")
                        nc.tensor.matmul(PEb[0:126, :], Eb[:, j * 126:(j + 1) * 126], EGs[g])
                        PN = pn.tile([126, NB], bf16, tag="pn", name="PN")
                        nc.vector.tensor_tensor(out=PN, in0=PEb[0:126, :],
                                                in1=VA[:, n, :], op=ALU.mult)
                        PNs.append(PN)
                        if n % 2 == 1:
                            nc.gpsimd.tensor_add(out=TST[:, :, n // 2],
                                                 in0=PNs[n - 1], in1=PNs[n])
                    CTXU = ph.tile([126, NB], f32, tag="ctxu", name="CTXU")
                    nc.vector.tensor_reduce(CTXU, TST[:, :, :], axis=AX.X, op=ALU.add)
                    CTX = ph.tile([126, NB], f32r, tag="ctx", name="CTX")
                    nc.vector.tensor_tensor(out=CTX, in0=CTXU, in1=PR[0:126, :], op=ALU.mult)

                    # ---- head MLP ----
                    PH1 = small.tile([128, NB], f32, tag="sm", name="PH1")
                    nc.tensor.matmul(PH1, Wat, CTX)
                    ATT = ph.tile([128, NB], f32r, tag="att", name="ATT")
                    nc.scalar.activation(ATT, PH1, AF.Tanh, bias=Bat)

                    PH2 = small.tile([128, NB], f32, tag="sm", name="PH2")
                    nc.tensor.matmul(PH2, Wop, OWN)
                    OWV = ph.tile([128, NB], f32r, tag="owv", name="OWV")
                    nc.scalar.activation(OWV, PH2, AF.Tanh, bias=Bop)

                    H1 = []
                    for mh in range(2):
                        PHh = small.tile([128, NB], f32, tag="sm", name="PHh")
                        nc.tensor.matmul(PHh, WH1[:, mh * 128:(mh + 1) * 128], OWV,
                                         start=True, stop=False)
                        nc.tensor.matmul(PHh, WH1[:, 256 + mh * 128:256 + (mh + 1) * 128], ATT,
                                         start=False, stop=True)
                        H1A = ph.tile([128, NB], f32r, tag=f"h1a{mh}", name="H1A")
                        nc.scalar.activation(H1A, PHh, AF.Prelu, bias=BH1[:, mh:mh + 1], alpha=ALPHA)
                        H1.append(H1A)
                    H2 = []
                    for mh in range(2):
                        PHh2 = small.tile([128, NB], f32, tag="sm", name="PHh2")
                        nc.tensor.matmul(PHh2, WH2[:, mh * 128:(mh + 1) * 128], H1[0],
                                         start=True, stop=False)
                        nc.tensor.matmul(PHh2, WH2[:, 256 + mh * 128:256 + (mh + 1) * 128], H1[1],
                                         start=False, stop=True)
                        H2A = ph.tile([128, NB], f32r, tag=f"h2a{mh}", name="H2A")
                        nc.scalar.activation(H2A, PHh2, AF.Prelu, bias=BH2[:, mh:mh + 1], alpha=ALPHA)
                        H2.append(H2A)

                    PO4 = small.tile([128, NB], f32, tag="sm", name="PO4")
                    nc.tensor.matmul(PO4[0:4, :], WOUT[:, 0:4], H2[0], start=True, stop=False)
                    nc.tensor.matmul(PO4[0:4, :], WOUT[:, 4:8], H2[1], start=False, stop=True)
                    OT = ph.tile([4, NB], f32, tag="ot", name="OT")
                    nc.scalar.activation(OT, PO4[0:4, :], AF.Identity, bias=Bout)
                    nc.sync.dma_start(out=out_d[:, bs:bs + NB], in_=OT)

    nc.compile()
    return nc


def _host_prep(inputs):
    """Build per-core input maps (numpy only)."""
    obs = np.ascontiguousarray(inputs["obs"], dtype=np.float32)
    w_own = np.asarray(inputs["w_own"], np.float32)
    w_int = np.asarray(inputs["w_int"], np.float32)
    wq = np.asarray(inputs["wq"], np.float32)
    wk = np.asarray(inputs["wk"], np.float32)
    wv = np.asarray(inputs["wv"], np.float32)
    v_att = np.asarray(inputs["v_att"], np.float32)
    w_attn = np.asarray(inputs["w_attn"], np.float32)
    w_ownp = np.asarray(inputs["w_ownp"], np.float32)
    w_h1 = np.asarray(inputs["w_h1"], np.float32)
    w_h2 = np.asarray(inputs["w_h2"], np.float32)
    w_out = np.asarray(inputs["w_out"], np.float32)

    def blockdiag(w):  # [H, D, D] -> [126, 126]
        out = np.zeros((TOT, TOT), np.float32)
        for h in range(H):
            out[h * D:(h + 1) * D, h * D:(h + 1) * D] = w[h]
        return out

    wia = np.zeros((126, 18 * 126), np.float32)
    for n in range(18):
        wia[7 * n:7 * n + 7, n * 126:(n + 1) * 126] = w_int
    wib = np.zeros((14, 2 * 126), np.float32)
    for n in range(2):
        wib[7 * n:7 * n + 7, n * 126:(n + 1) * 126] = w_int

    va32 = np.zeros((126, 32), np.float32)
    for h in range(H):
        va32[h * D:(h + 1) * D, h] = v_att[h]

    densel = np.zeros((128, 3), np.float32)
    for j in range(4):
        for h in range(H):
            densel[32 * j + h, h] = 1.0

    ebcsel = np.zeros((128, 4 * 126), np.float32)
    for j in range(4):
        for h in range(H):
            ebcsel[32 * j + h, j * 126 + h * D:(j * 126) + (h + 1) * D] = 1.0

    rbc = np.zeros((3, 126), np.float32)
    for h in range(H):
        rbc[h, h * D:(h + 1) * D] = 1.0

    wh1r = np.ascontiguousarray(
        w_h1.reshape(2, 128, HID).transpose(1, 0, 2).reshape(128, 512))
    wh2r = np.ascontiguousarray(
        w_h2.reshape(2, 128, HID).transpose(1, 0, 2).reshape(128, 512))
    woutr = np.ascontiguousarray(
        w_out.reshape(2, 128, NOUT).transpose(1, 0, 2).reshape(128, 8))

    params = {
        "wia": wia, "wib": wib, "wo": w_own,
        "wqb": blockdiag(wq), "wkb": blockdiag(wk), "wvb": blockdiag(wv),
        "va32": va32.astype(_bf16np), "densel": densel, "ebcsel": ebcsel, "rbc": rbc,
        "wat": w_attn, "wop": w_ownp,
        "wh1r": wh1r, "wh2r": wh2r, "woutr": woutr,
        "bown": np.asarray(inputs["b_own"], np.float32).reshape(126, 1),
        "bint": np.asarray(inputs["b_int"], np.float32).reshape(126, 1),
        "bat": np.asarray(inputs["b_attn"], np.float32).reshape(128, 1),
        "bop": np.asarray(inputs["b_ownp"], np.float32).reshape(128, 1),
        "bh1": np.ascontiguousarray(
            np.asarray(inputs["b_h1"], np.float32).reshape(2, 128).T),
        "bh2": np.ascontiguousarray(
            np.asarray(inputs["b_h2"], np.float32).reshape(2, 128).T),
        "bout": np.asarray(inputs["b_out"], np.float32).reshape(4, 1),
    }

    in_maps = []
    for c in range(N_CORES):
        sl = obs[c * BC:(c + 1) * BC]                       # [BC, 147]
        xo = np.ascontiguousarray(sl[:, :OWN_DIM].T)        # [7, BC]
        intr = sl[:, OWN_DIM:].reshape(BC, N_INTR, INT_DIM)  # [BC, 20, 7]
        intrT = intr.transpose(1, 2, 0)                     # [20, 7, BC]
        xa = np.ascontiguousarray(intrT[:18].reshape(126, BC))
        xb = np.ascontiguousarray(intrT[18:].reshape(14, BC))
        m = {"xo": xo, "xa": xa, "xb": xb}
        m.update(params)
        in_maps.append(m)
    return in_maps


def _get_nc():
    if "nc" not in _BUILT:
        _BUILT["nc"] = _build_nc()
    return _BUILT["nc"]


def run(inputs, trace=False):
    from concourse.bass_utils import run_bass_kernel_spmd
    nc = _get_nc()
    in_maps = _host_prep(inputs)
    res = run_bass_kernel_spmd(nc, in_maps, core_ids=list(range(N_CORES)),
                               trace=trace)
    outs = [res.results[c]["outT"] for c in range(N_CORES)]   # each [4, BC]
    full = np.concatenate(outs, axis=1).T                     # [B, 4]
    return np.ascontiguousarray(full, dtype=np.float32), res


def kernel(**inputs):
    out, _ = run(inputs, trace=False)
    return out


# revision 5
# speedup vs baseline: 1.4919x; 1.0318x over previous
"""Trainium2 Bass kernel for nn_AttentionSACModel (sparse_attention).

Data-parallel across 8 NeuronCores: obs sharded along batch, params replicated.
On-device layout keeps batch on the matmul free dim (activations stored
feature-major / transposed); all host<->device layout changes happen in numpy.
"""
import sys
import os

if "/opt/trn_rl_repo" not in sys.path:
    sys.path.insert(0, "/opt/trn_rl_repo")

import numpy as np
import ml_dtypes
_bf16np = ml_dtypes.bfloat16

OWN_DIM = 7
INT_DIM = 7
N_INTR = 20
H = 3
D = 42
TOT = H * D            # 126
ATTN = 128
HID = 256
NOUT = 4
B = 32768
N_CORES = 8
BC = B // N_CORES      # 4096 rows per core
NB = 512               # batch tile (matmul free dim)
NT = BC // NB          # 8 tiles per core
ALPHA = 0.2            # leaky relu slope

_BUILT = {}


def _build_nc():
    import concourse.bacc as bacc
    import concourse.tile as tile
    from concourse import mybir

    f32 = mybir.dt.float32
    f32r = mybir.dt.float32r
    bf16 = mybir.dt.bfloat16
    AF = mybir.ActivationFunctionType
    ALU = mybir.AluOpType
    AX = mybir.AxisListType

    nc = bacc.Bacc()

    # ---- DRAM I/O ----
    xo_d = nc.dram_tensor("xo", [OWN_DIM, BC], f32r, kind="ExternalInput")
    xa_d = nc.dram_tensor("xa", [126, BC], f32r, kind="ExternalInput")       # interactors 0..17, row 7n+f
    xb_d = nc.dram_tensor("xb", [14, BC], f32r, kind="ExternalInput")        # interactors 18,19
    wia_d = nc.dram_tensor("wia", [126, 18 * 126], f32r, kind="ExternalInput")  # padded int-embed lhsT, n<18
    wib_d = nc.dram_tensor("wib", [14, 2 * 126], f32r, kind="ExternalInput")    # n=18,19
    wo_d = nc.dram_tensor("wo", [7, 126], f32r, kind="ExternalInput")
    wq_d = nc.dram_tensor("wqb", [126, 126], f32r, kind="ExternalInput")
    wk_d = nc.dram_tensor("wkb", [126, 126], f32r, kind="ExternalInput")
    wv_d = nc.dram_tensor("wvb", [126, 126], f32r, kind="ExternalInput")
    va_d = nc.dram_tensor("va32", [126, 32], bf16, kind="ExternalInput")
    ds_d = nc.dram_tensor("densel", [128, 3], f32r, kind="ExternalInput")
    eb_d = nc.dram_tensor("ebcsel", [128, 4 * 126], f32r, kind="ExternalInput")
    rb_d = nc.dram_tensor("rbc", [3, 126], f32r, kind="ExternalInput")
    wat_d = nc.dram_tensor("wat", [126, 128], f32r, kind="ExternalInput")
    wop_d = nc.dram_tensor("wop", [126, 128], f32r, kind="ExternalInput")
    wh1_d = nc.dram_tensor("wh1r", [128, 512], f32r, kind="ExternalInput")   # [p, kc*256+m]
    wh2_d = nc.dram_tensor("wh2r", [128, 512], f32r, kind="ExternalInput")
    wout_d = nc.dram_tensor("woutr", [128, 8], f32r, kind="ExternalInput")   # [p, kc*4+m]
    bown_d = nc.dram_tensor("bown", [126, 1], f32, kind="ExternalInput")
    bint_d = nc.dram_tensor("bint", [126, 1], f32, kind="ExternalInput")
    bat_d = nc.dram_tensor("bat", [128, 1], f32, kind="ExternalInput")
    bop_d = nc.dram_tensor("bop", [128, 1], f32, kind="ExternalInput")
    bh1_d = nc.dram_tensor("bh1", [128, 2], f32, kind="ExternalInput")
    bh2_d = nc.dram_tensor("bh2", [128, 2], f32, kind="ExternalInput")
    bout_d = nc.dram_tensor("bout", [4, 1], f32, kind="ExternalInput")
    out_d = nc.dram_tensor("outT", [NOUT, BC], f32, kind="ExternalOutput")

    with tile.TileContext(nc) as tc:
        with tc.tile_pool(name="const", bufs=1) as cst, \
             tc.tile_pool(name="px", bufs=2) as px, \
             tc.tile_pool(name="pemb", bufs=3) as pemb, \
             tc.tile_pool(name="peng", bufs=3) as peng, \
             tc.tile_pool(name="pE", bufs=10) as pE, \
             tc.tile_pool(name="pv", bufs=2) as pv, \
             tc.tile_pool(name="pp", bufs=2) as pp, \
             tc.tile_pool(name="pn", bufs=6) as pn, \
             tc.tile_pool(name="ph", bufs=2) as ph, \
             tc.tile_pool(name="pz", bufs=2, space="PSUM") as ppz, \
             tc.tile_pool(name="pk", bufs=2, space="PSUM") as ppk, \
             tc.tile_pool(name="sm", bufs=3, space="PSUM") as small, \
             tc.tile_pool(name="pd", bufs=1, space="PSUM") as ppd:

            # ---- load constants ----
            WiA = cst.tile([126, 18 * 126], f32r)
            WiB = cst.tile([14, 2 * 126], f32r)
            Wo = cst.tile([7, 126], f32r)
            Wq = cst.tile([126, 126], f32r)
            Wk = cst.tile([126, 126], f32r)
            Wv = cst.tile([126, 126], f32r)
            Va = cst.tile([126, 32], bf16)
            Ds = cst.tile([128, 3], f32r)
            Eb = cst.tile([128, 4 * 126], f32r)
            Rb = cst.tile([3, 126], f32r)
            Wat = cst.tile([126, 128], f32r)
            Wop = cst.tile([126, 128], f32r)
            WH1 = cst.tile([128, 512], f32r)
            WH2 = cst.tile([128, 512], f32r)
            WOUT = cst.tile([128, 8], f32r)
            Bown = cst.tile([126, 1], f32)
            Bint = cst.tile([126, 1], f32)
            Bat = cst.tile([128, 1], f32)
            Bop = cst.tile([128, 1], f32)
            BH1 = cst.tile([128, 2], f32)
            BH2 = cst.tile([128, 2], f32)
            Bout = cst.tile([4, 1], f32)
            for t_sb, t_dr in [(WiA, wia_d), (WiB, wib_d), (Wo, wo_d), (Wq, wq_d),
                               (Wk, wk_d), (Wv, wv_d), (Va, va_d), (Ds, ds_d),
                               (Eb, eb_d), (Rb, rb_d), (Wat, wat_d), (Wop, wop_d),
                               (WH1, wh1_d), (WH2, wh2_d), (WOUT, wout_d),
                               (Bown, bown_d), (Bint, bint_d), (Bat, bat_d),
                               (Bop, bop_d), (BH1, bh1_d), (BH2, bh2_d),
                               (Bout, bout_d)]:
                nc.sync.dma_start(out=t_sb, in_=t_dr[:, :])

            with nc.allow_low_precision(reason="bf16/f32r intermediates; final accums are f32"):
                state = {}

                def phase_b(t):
                    """embed + k/q/v + scores + exp for tile t"""
                    bs = t * NB
                    XO = px.tile([OWN_DIM, NB], f32r, tag="xo", name="XO")
                    XA = px.tile([126, NB], f32r, tag="xa", name="XA")
                    XB = px.tile([14, NB], f32r, tag="xb", name="XB")
                    nc.sync.dma_start(out=XO, in_=xo_d[:, bs:bs + NB])
                    nc.sync.dma_start(out=XA, in_=xa_d[:, bs:bs + NB])
                    nc.sync.dma_start(out=XB, in_=xb_d[:, bs:bs + NB])

                    PO = small.tile([128, NB], f32, tag="sm", name="PO")
                    nc.tensor.matmul(PO[0:126, :], Wo, XO)
                    OWN = ph.tile([126, NB], f32r, tag="own", name="OWN")
                    nc.scalar.activation(OWN, PO[0:126, :], AF.Prelu, bias=Bown, alpha=ALPHA)

                    EGs = []
                    VA = pv.tile([126, N_INTR, NB], bf16, tag="va", name="VA")
                    PS = None
                    for n in range(N_INTR):
                        PZ = ppz.tile([126, NB], f32, tag="pz", name="PZ")
                        if n < 18:
                            nc.tensor.matmul(PZ, WiA[:, n * 126:(n + 1) * 126], XA)
                        else:
                            nc.tensor.matmul(PZ, WiB[:, (n - 18) * 126:(n - 17) * 126], XB)
                        ZT = pemb.tile([126, NB], f32r, tag="zt", name="ZT")
                        nc.scalar.activation(ZT, PZ, AF.Prelu, bias=Bint, alpha=ALPHA)

                        PK = ppk.tile([126, NB], f32, tag="pk", name="PK")
                        nc.tensor.matmul(PK, Wk, ZT, start=True, stop=False)
                        nc.tensor.matmul(PK, Wq, OWN, start=False, stop=True)
                        EN = peng.tile([126, NB], bf16, tag="en", name="EN")
                        nc.scalar.activation(EN, PK, AF.Tanh)

                        PV = small.tile([128, NB], f32, tag="sm", name="PV")
                        nc.tensor.matmul(PV[0:126, :], Wv, ZT)
                        nc.scalar.activation(VA[:, n, :], PV[0:126, :], AF.Copy)

                        j = n % 4
                        if j == 0:
                            PS = small.tile([128, NB], f32, tag="sm", name="PS")
                        nc.tensor.matmul(PS[32 * j:32 * (j + 1), :], Va, EN,
                                         tile_position=(0, 32 * j))
                        if j == 3:
                            EG = pE.tile([128, NB], f32r, tag="eg", name="EG")
                            nc.scalar.activation(EG, PS, AF.Exp)
                            EGs.append(EG)
                    state[t] = {"OWN": OWN, "VA": VA, "EGs": EGs}

                def phase_cd(t):
                    """softmax denom + ctx + head MLP + output for tile t"""
                    bs = t * NB
                    OWN = state[t]["OWN"]
                    VA = state[t]["VA"]
                    EGs = state[t]["EGs"]

                    PD = ppd.tile([128, NB], f32, tag="pd", name="PD")
                    for g in range(5):
                        nc.tensor.matmul(PD[0:3, :], Ds, EGs[g],
                                         start=(g == 0), stop=(g == 4))
                    RD = ph.tile([3, NB], f32r, tag="rd", name="RD")
                    nc.vector.reciprocal(RD, PD[0:3, :])
                    PR = small.tile([128, NB], f32, tag="sm", name="PR")
                    nc.tensor.matmul(PR[0:126, :], Rb, RD)

                    TST = pp.tile([126, NB, N_INTR // 2], bf16, tag="tst", name="TST")
                    PNs = []
                    for n in range(N_INTR):
                        g, j = n // 4, n % 4
                        PEb = small.tile([128, NB], f32, tag="sm", name="PEb")
                        nc.tensor.matmul(PEb[0:126, :], Eb[:, j * 126:(j + 1) * 126], EGs[g])
                        PN = pn.tile([126, NB], bf16, tag="pn", name="PN")
                        nc.vector.tensor_tensor(out=PN, in0=PEb[0:126, :],
                                                in1=VA[:, n, :], op=ALU.mult)
                        PNs.append(PN)
                        if n % 2 == 1:
                            nc.gpsimd.tensor_add(out=TST[:, :, n // 2],
                                                 in0=PNs[n - 1], in1=PNs[n])
                    CTXU = ph.tile([126, NB], f32, tag="ctxu", name="CTXU")
                    nc.vector.tensor_reduce(CTXU, TST[:, :, :], axis=AX.X, op=ALU.add)
                    CTX = ph.tile([126, NB], f32r, tag="ctx", name="CTX")
                    nc.vector.tensor_tensor(out=CTX, in0=CTXU, in1=PR[0:126, :], op=ALU.mult)

                    PH1 = small.tile([128, NB], f32, tag="sm", name="PH1")
                    nc.tensor.matmul(PH1, Wat, CTX)
                    ATT = ph.tile([128, NB], f32r, tag="att", name="ATT")
                    nc.scalar.activation(ATT, PH1, AF.Tanh, bias=Bat)

                    PH2 = small.tile([128, NB], f32, tag="sm", name="PH2")
                    nc.tensor.matmul(PH2, Wop, OWN)
                    OWV = ph.tile([128, NB], f32r, tag="owv", name="OWV")
                    nc.scalar.activation(OWV, PH2, AF.Tanh, bias=Bop)

                    H1 = []
                    for mh in range(2):
                        PHh = small.tile([128, NB], f32, tag="sm", name="PHh")
                        nc.tensor.matmul(PHh, WH1[:, mh * 128:(mh + 1) * 128], OWV,
                                         start=True, stop=False)
                        nc.tensor.matmul(PHh, WH1[:, 256 + mh * 128:256 + (mh + 1) * 128], ATT,
                                         start=False, stop=True)
                        H1A = ph.tile([128, NB], f32r, tag=f"h1a{mh}", name="H1A")
                        nc.scalar.activation(H1A, PHh, AF.Prelu, bias=BH1[:, mh:mh + 1], alpha=ALPHA)
                        H1.append(H1A)
                    H2 = []
                    for mh in range(2):
                        PHh2 = small.tile([128, NB], f32, tag="sm", name="PHh2")
                        nc.tensor.matmul(PHh2, WH2[:, mh * 128:(mh + 1) * 128], H1[0],
                                         start=True, stop=False)
                        nc.tensor.matmul(PHh2, WH2[:, 256 + mh * 128:256 + (mh + 1) * 128], H1[1],
                                         start=False, stop=True)
                        H2A = ph.tile([128, NB], f32r, tag=f"h2a{mh}", name="H2A")
                        nc.scalar.activation(H2A, PHh2, AF.Prelu, bias=BH2[:, mh:mh + 1], alpha=ALPHA)
                        H2.append(H2A)

                    PO4 = small.tile([128, NB], f32, tag="sm", name="PO4")
                    nc.tensor.matmul(PO4[0:4, :], WOUT[:, 0:4], H2[0], start=True, stop=False)
                    nc.tensor.matmul(PO4[0:4, :], WOUT[:, 4:8], H2[1], start=False, stop=True)
                    OT = ph.tile([4, NB], f32, tag="ot", name="OT")
                    nc.scalar.activation(OT, PO4[0:4, :], AF.Identity, bias=Bout)
                    nc.sync.dma_start(out=out_d[:, bs:bs + NB], in_=OT)
                    del state[t]

                # software pipeline: phase B of tile t runs ahead of phase C/D of t-1
                phase_b(0)
                for t in range(1, NT):
                    phase_b(t)
                    phase_cd(t - 1)
                phase_cd(NT - 1)

    nc.compile()
    return nc


def _host_prep(inputs):
    """Build per-core input maps (numpy only)."""
    obs = np.ascontiguousarray(inputs["obs"], dtype=np.float32)
    w_own = np.asarray(inputs["w_own"], np.float32)
    w_int = np.asarray(inputs["w_int"], np.float32)
    wq = np.asarray(inputs["wq"], np.float32)
    wk = np.asarray(inputs["wk"], np.float32)
    wv = np.asarray(inputs["wv"], np.float32)
    v_att = np.asarray(inputs["v_att"], np.float32)
    w_attn = np.asarray(inputs["w_attn"], np.float32)
    w_ownp = np.asarray(inputs["w_ownp"], np.float32)
    w_h1 = np.asarray(inputs["w_h1"], np.float32)
    w_h2 = np.asarray(inputs["w_h2"], np.float32)
    w_out = np.asarray(inputs["w_out"], np.float32)

    def blockdiag(w):  # [H, D, D] -> [126, 126]
        out = np.zeros((TOT, TOT), np.float32)
        for h in range(H):
            out[h * D:(h + 1) * D, h * D:(h + 1) * D] = w[h]
        return out

    wia = np.zeros((126, 18 * 126), np.float32)
    for n in range(18):
        wia[7 * n:7 * n + 7, n * 126:(n + 1) * 126] = w_int
    wib = np.zeros((14, 2 * 126), np.float32)
    for n in range(2):
        wib[7 * n:7 * n + 7, n * 126:(n + 1) * 126] = w_int

    va32 = np.zeros((126, 32), np.float32)
    for h in range(H):
        va32[h * D:(h + 1) * D, h] = v_att[h]

    densel = np.zeros((128, 3), np.float32)
    for j in range(4):
        for h in range(H):
            densel[32 * j + h, h] = 1.0

    ebcsel = np.zeros((128, 4 * 126), np.float32)
    for j in range(4):
        for h in range(H):
            ebcsel[32 * j + h, j * 126 + h * D:(j * 126) + (h + 1) * D] = 1.0

    rbc = np.zeros((3, 126), np.float32)
    for h in range(H):
        rbc[h, h * D:(h + 1) * D] = 1.0

    wh1r = np.ascontiguousarray(
        w_h1.reshape(2, 128, HID).transpose(1, 0, 2).reshape(128, 512))
    wh2r = np.ascontiguousarray(
        w_h2.reshape(2, 128, HID).transpose(1, 0, 2).reshape(128, 512))
    woutr = np.ascontiguousarray(
        w_out.reshape(2, 128, NOUT).transpose(1, 0, 2).reshape(128, 8))

    params = {
        "wia": wia, "wib": wib, "wo": w_own,
        "wqb": blockdiag(wq), "wkb": blockdiag(wk), "wvb": blockdiag(wv),
        "va32": va32.astype(_bf16np), "densel": densel, "ebcsel": ebcsel, "rbc": rbc,
        "wat": w_attn, "wop": w_ownp,
        "wh1r": wh1r, "wh2r": wh2r, "woutr": woutr,
        "bown": np.asarray(inputs["b_own"], np.float32).reshape(126, 1),
        "bint": np.asarray(inputs["b_int"], np.float32).reshape(126, 1),
        "bat": np.asarray(inputs["b_attn"], np.float32).reshape(128, 1),
        "bop": np.asarray(inputs["b_ownp"], np.float32).reshape(128, 1),
        "bh1": np.ascontiguousarray(
            np.asarray(inputs["b_h1"], np.float32).reshape(2, 128).T),
        "bh2": np.ascontiguousarray(
            np.asarray(inputs["b_h2"], np.float32).reshape(2, 128).T),
        "bout": np.asarray(inputs["b_out"], np.float32).reshape(4, 1),
    }

    in_maps = []
    for c in range(N_CORES):
        sl = obs[c * BC:(c + 1) * BC]                       # [BC, 147]
        xo = np.ascontiguousarray(sl[:, :OWN_DIM].T)        # [7, BC]
        intr = sl[:, OWN_DIM:].reshape(BC, N_INTR, INT_DIM)  # [BC, 20, 7]
        intrT = intr.transpose(1, 2, 0)                     # [20, 7, BC]
        xa = np.ascontiguousarray(intrT[:18].reshape(126, BC))
        xb = np.ascontiguousarray(intrT[18:].reshape(14, BC))
        m = {"xo": xo, "xa": xa, "xb": xb}
        m.update(params)
        in_maps.append(m)
    return in_maps


def _get_nc():
    if "nc" not in _BUILT:
        _BUILT["nc"] = _build_nc()
    return _BUILT["nc"]


def run(inputs, trace=False):
    from concourse.bass_utils import run_bass_kernel_spmd
    nc = _get_nc()
    in_maps = _host_prep(inputs)
    res = run_bass_kernel_spmd(nc, in_maps, core_ids=list(range(N_CORES)),
                               trace=trace)
    outs = [res.results[c]["outT"] for c in range(N_CORES)]   # each [4, BC]
    full = np.concatenate(outs, axis=1).T                     # [B, 4]
    return np.ascontiguousarray(full, dtype=np.float32), res


def kernel(**inputs):
    out, _ = run(inputs, trace=False)
    return out


# revision 6
# speedup vs baseline: 1.8062x; 1.2107x over previous
"""Trainium2 Bass kernel for nn_AttentionSACModel (sparse_attention).

Data-parallel across 8 NeuronCores: obs sharded along batch, params replicated.
On-device layout keeps batch on the matmul free dim (activations stored
feature-major / transposed); all host<->device layout changes happen in numpy.
"""
import sys
import os

if "/opt/trn_rl_repo" not in sys.path:
    sys.path.insert(0, "/opt/trn_rl_repo")

import numpy as np
import ml_dtypes
_bf16np = ml_dtypes.bfloat16

OWN_DIM = 7
INT_DIM = 7
N_INTR = 20
H = 3
D = 42
TOT = H * D            # 126
ATTN = 128
HID = 256
NOUT = 4
B = 32768
N_CORES = 8
BC = B // N_CORES      # 4096 rows per core
NB = 512               # batch tile (matmul free dim)
NT = BC // NB          # 8 tiles per core
ALPHA = 0.2            # leaky relu slope

_BUILT = {}


def _build_nc():
    import concourse.bacc as bacc
    import concourse.tile as tile
    from concourse import mybir

    f32 = mybir.dt.float32
    f32r = mybir.dt.float32r
    bf16 = mybir.dt.bfloat16
    AF = mybir.ActivationFunctionType
    ALU = mybir.AluOpType
    AX = mybir.AxisListType

    nc = bacc.Bacc()

    # ---- DRAM I/O ----
    xo_d = nc.dram_tensor("xo", [OWN_DIM, BC], f32r, kind="ExternalInput")
    xa_d = nc.dram_tensor("xa", [126, BC], f32r, kind="ExternalInput")       # interactors 0..17, row 7n+f
    xb_d = nc.dram_tensor("xb", [14, BC], f32r, kind="ExternalInput")        # interactors 18,19
    wia_d = nc.dram_tensor("wia", [126, 18 * 126], f32r, kind="ExternalInput")  # padded int-embed lhsT, n<18
    wib_d = nc.dram_tensor("wib", [14, 2 * 126], f32r, kind="ExternalInput")    # n=18,19
    wo_d = nc.dram_tensor("wo", [7, 126], f32r, kind="ExternalInput")
    wq_d = nc.dram_tensor("wqb", [126, 126], f32r, kind="ExternalInput")
    wk_d = nc.dram_tensor("wkb", [126, 126], f32r, kind="ExternalInput")
    wv_d = nc.dram_tensor("wvb", [126, 126], f32r, kind="ExternalInput")
    va_d = nc.dram_tensor("va32", [126, 32], bf16, kind="ExternalInput")
    ds_d = nc.dram_tensor("densel", [128, 3], f32r, kind="ExternalInput")
    eb_d = nc.dram_tensor("ebcsel", [128, 4 * 126], f32r, kind="ExternalInput")
    rb_d = nc.dram_tensor("rbc", [3, 126], f32r, kind="ExternalInput")
    wat_d = nc.dram_tensor("wat", [126, 128], f32r, kind="ExternalInput")
    wop_d = nc.dram_tensor("wop", [126, 128], f32r, kind="ExternalInput")
    wh1_d = nc.dram_tensor("wh1r", [128, 512], f32r, kind="ExternalInput")   # [p, kc*256+m]
    wh2_d = nc.dram_tensor("wh2r", [128, 512], f32r, kind="ExternalInput")
    wout_d = nc.dram_tensor("woutr", [128, 8], f32r, kind="ExternalInput")   # [p, kc*4+m]
    bown_d = nc.dram_tensor("bown", [126, 1], f32, kind="ExternalInput")
    bint_d = nc.dram_tensor("bint", [126, 1], f32, kind="ExternalInput")
    bat_d = nc.dram_tensor("bat", [128, 1], f32, kind="ExternalInput")
    bop_d = nc.dram_tensor("bop", [128, 1], f32, kind="ExternalInput")
    bh1_d = nc.dram_tensor("bh1", [128, 2], f32, kind="ExternalInput")
    bh2_d = nc.dram_tensor("bh2", [128, 2], f32, kind="ExternalInput")
    bout_d = nc.dram_tensor("bout", [4, 1], f32, kind="ExternalInput")
    out_d = nc.dram_tensor("outT", [NOUT, BC], f32, kind="ExternalOutput")

    with tile.TileContext(nc) as tc:
        with tc.tile_pool(name="const", bufs=1) as cst, \
             tc.tile_pool(name="px", bufs=2) as px, \
             tc.tile_pool(name="pemb", bufs=3) as pemb, \
             tc.tile_pool(name="peng", bufs=3) as peng, \
             tc.tile_pool(name="pE", bufs=10) as pE, \
             tc.tile_pool(name="pv", bufs=2) as pv, \
             tc.tile_pool(name="pp", bufs=2) as pp, \
             tc.tile_pool(name="pn", bufs=6) as pn, \
             tc.tile_pool(name="ph", bufs=2) as ph, \
             tc.tile_pool(name="pz", bufs=2, space="PSUM") as ppz, \
             tc.tile_pool(name="pk", bufs=2, space="PSUM") as ppk, \
             tc.tile_pool(name="sm", bufs=3, space="PSUM") as small, \
             tc.tile_pool(name="pd", bufs=1, space="PSUM") as ppd:

            # ---- load constants ----
            WiA = cst.tile([126, 18 * 126], f32r)
            WiB = cst.tile([14, 2 * 126], f32r)
            Wo = cst.tile([7, 126], f32r)
            Wq = cst.tile([126, 126], f32r)
            Wk = cst.tile([126, 126], f32r)
            Wv = cst.tile([126, 126], f32r)
            Va = cst.tile([126, 32], bf16)
            Ds = cst.tile([128, 3], f32r)
            Eb = cst.tile([128, 4 * 126], f32r)
            Rb = cst.tile([3, 126], f32r)
            Wat = cst.tile([126, 128], f32r)
            Wop = cst.tile([126, 128], f32r)
            WH1 = cst.tile([128, 512], f32r)
            WH2 = cst.tile([128, 512], f32r)
            WOUT = cst.tile([128, 8], f32r)
            Bown = cst.tile([126, 1], f32)
            Bint = cst.tile([126, 1], f32)
            Bat = cst.tile([128, 1], f32)
            Bop = cst.tile([128, 1], f32)
            BH1 = cst.tile([128, 2], f32)
            BH2 = cst.tile([128, 2], f32)
            Bout = cst.tile([4, 1], f32)
            for t_sb, t_dr in [(WiA, wia_d), (WiB, wib_d), (Wo, wo_d), (Wq, wq_d),
                               (Wk, wk_d), (Wv, wv_d), (Va, va_d), (Ds, ds_d),
                               (Eb, eb_d), (Rb, rb_d), (Wat, wat_d), (Wop, wop_d),
                               (WH1, wh1_d), (WH2, wh2_d), (WOUT, wout_d),
                               (Bown, bown_d), (Bint, bint_d), (Bat, bat_d),
                               (Bop, bop_d), (BH1, bh1_d), (BH2, bh2_d),
                               (Bout, bout_d)]:
                nc.sync.dma_start(out=t_sb, in_=t_dr[:, :])

            with nc.allow_low_precision(reason="bf16/f32r intermediates; final accums are f32"):
                state = {}

                def phase_b(t):
                    """embed + k/q/v + scores + exp for tile t"""
                    bs = t * NB
                    XO = px.tile([OWN_DIM, NB], f32r, tag="xo", name="XO")
                    XA = px.tile([126, NB], f32r, tag="xa", name="XA")
                    XB = px.tile([14, NB], f32r, tag="xb", name="XB")
                    nc.sync.dma_start(out=XO, in_=xo_d[:, bs:bs + NB])
                    nc.sync.dma_start(out=XA, in_=xa_d[:, bs:bs + NB])
                    nc.sync.dma_start(out=XB, in_=xb_d[:, bs:bs + NB])

                    PO = small.tile([128, NB], f32, tag="sm", name="PO")
                    nc.tensor.matmul(PO[0:126, :], Wo, XO)
                    OWN = ph.tile([126, NB], f32r, tag="own", name="OWN")
                    nc.scalar.activation(OWN, PO[0:126, :], AF.Prelu, bias=Bown, alpha=ALPHA)

                    EGs = []
                    VA = pv.tile([126, N_INTR, NB], bf16, tag="va", name="VA")
                    PS = None
                    ZTs = {}

                    def emit_z(n):
                        PZ = ppz.tile([126, NB], f32, tag="pz", name="PZ")
                        if n < 18:
                            nc.tensor.matmul(PZ, WiA[:, n * 126:(n + 1) * 126], XA)
                        else:
                            nc.tensor.matmul(PZ, WiB[:, (n - 18) * 126:(n - 17) * 126], XB)
                        ZT = pemb.tile([126, NB], f32r, tag="zt", name="ZT")
                        nc.scalar.activation(ZT, PZ, AF.Prelu, bias=Bint, alpha=ALPHA)
                        ZTs[n] = ZT

                    emit_z(0)
                    emit_z(1)
                    for n in range(N_INTR):
                        ZT = ZTs.pop(n)
                        PK = ppk.tile([126, NB], f32, tag="pk", name="PK")
                        nc.tensor.matmul(PK, Wk, ZT, start=True, stop=False)
                        nc.tensor.matmul(PK, Wq, OWN, start=False, stop=True)
                        EN = peng.tile([126, NB], bf16, tag="en", name="EN")
                        nc.scalar.activation(EN, PK, AF.Tanh)

                        PV = small.tile([128, NB], f32, tag="sm", name="PV")
                        nc.tensor.matmul(PV[0:126, :], Wv, ZT)
                        nc.scalar.activation(VA[:, n, :], PV[0:126, :], AF.Copy)

                        if n + 2 < N_INTR:
                            emit_z(n + 2)

                        j = n % 4
                        if j == 0:
                            PS = small.tile([128, NB], f32, tag="sm", name="PS")
                        nc.tensor.matmul(PS[32 * j:32 * (j + 1), :], Va, EN,
                                         tile_position=(0, 32 * j))
                        if j == 3:
                            EG = pE.tile([128, NB], f32r, tag="eg", name="EG")
                            nc.scalar.activation(EG, PS, AF.Exp)
                            EGs.append(EG)
                    state[t] = {"OWN": OWN, "VA": VA, "EGs": EGs}

                def phase_cd1(t):
                    """softmax denom + ctx for tile t"""
                    VA = state[t]["VA"]
                    EGs = state[t]["EGs"]

                    PD = ppd.tile([128, NB], f32, tag="pd", name="PD")
                    for g in range(5):
                        nc.tensor.matmul(PD[0:3, :], Ds, EGs[g],
                                         start=(g == 0), stop=(g == 4))
                    RD = ph.tile([3, NB], f32r, tag="rd", name="RD")
                    nc.vector.reciprocal(RD, PD[0:3, :])
                    PR = small.tile([128, NB], f32, tag="sm", name="PR")
                    nc.tensor.matmul(PR[0:126, :], Rb, RD)

                    TST = pp.tile([126, NB, N_INTR // 2], bf16, tag="tst", name="TST")
                    PNs = []
                    for n in range(N_INTR):
                        g, j = n // 4, n % 4
                        PEb = small.tile([128, NB], f32, tag="sm", name="PEb")
                        nc.tensor.matmul(PEb[0:126, :], Eb[:, j * 126:(j + 1) * 126], EGs[g])
                        PN = pn.tile([126, NB], bf16, tag="pn", name="PN")
                        nc.vector.tensor_tensor(out=PN, in0=PEb[0:126, :],
                                                in1=VA[:, n, :], op=ALU.mult)
                        PNs.append(PN)
                        if n % 2 == 1:
                            nc.gpsimd.tensor_add(out=TST[:, :, n // 2],
                                                 in0=PNs[n - 1], in1=PNs[n])
                    CTXU = ph.tile([126, NB], f32, tag="ctxu", name="CTXU")
                    nc.vector.tensor_reduce(CTXU, TST[:, :, :], axis=AX.X, op=ALU.add)
                    CTX = ph.tile([126, NB], f32r, tag="ctx", name="CTX")
                    nc.vector.tensor_tensor(out=CTX, in0=CTXU, in1=PR[0:126, :], op=ALU.mult)
                    state[t]["CTX"] = CTX

                def phase_cd2(t):
                    """head MLP + output for tile t"""
                    bs = t * NB
                    OWN = state[t]["OWN"]
                    CTX = state[t]["CTX"]

                    PH1 = small.tile([128, NB], f32, tag="sm", name="PH1")
                    nc.tensor.matmul(PH1, Wat, CTX)
                    ATT = ph.tile([128, NB], f32r, tag="att", name="ATT")
                    nc.scalar.activation(ATT, PH1, AF.Tanh, bias=Bat)

                    PH2 = small.tile([128, NB], f32, tag="sm", name="PH2")
                    nc.tensor.matmul(PH2, Wop, OWN)
                    OWV = ph.tile([128, NB], f32r, tag="owv", name="OWV")
                    nc.scalar.activation(OWV, PH2, AF.Tanh, bias=Bop)

                    H1 = []
                    for mh in range(2):
                        PHh = small.tile([128, NB], f32, tag="sm", name="PHh")
                        nc.tensor.matmul(PHh, WH1[:, mh * 128:(mh + 1) * 128], OWV,
                                         start=True, stop=False)
                        nc.tensor.matmul(PHh, WH1[:, 256 + mh * 128:256 + (mh + 1) * 128], ATT,
                                         start=False, stop=True)
                        H1A = ph.tile([128, NB], f32r, tag=f"h1a{mh}", name="H1A")
                        nc.scalar.activation(H1A, PHh, AF.Prelu, bias=BH1[:, mh:mh + 1], alpha=ALPHA)
                        H1.append(H1A)
                    H2 = []
                    for mh in range(2):
                        PHh2 = small.tile([128, NB], f32, tag="sm", name="PHh2")
                        nc.tensor.matmul(PHh2, WH2[:, mh * 128:(mh + 1) * 128], H1[0],
                                         start=True, stop=False)
                        nc.tensor.matmul(PHh2, WH2[:, 256 + mh * 128:256 + (mh + 1) * 128], H1[1],
                                         start=False, stop=True)
                        H2A = ph.tile([128, NB], f32r, tag=f"h2a{mh}", name="H2A")
                        nc.scalar.activation(H2A, PHh2, AF.Prelu, bias=BH2[:, mh:mh + 1], alpha=ALPHA)
                        H2.append(H2A)

                    PO4 = small.tile([128, NB], f32, tag="sm", name="PO4")
                    nc.tensor.matmul(PO4[0:4, :], WOUT[:, 0:4], H2[0], start=True, stop=False)
                    nc.tensor.matmul(PO4[0:4, :], WOUT[:, 4:8], H2[1], start=False, stop=True)
                    OT = ph.tile([4, NB], f32, tag="ot", name="OT")
                    nc.scalar.activation(OT, PO4[0:4, :], AF.Identity, bias=Bout)
                    nc.sync.dma_start(out=out_d[:, bs:bs + NB], in_=OT)
                    del state[t]

                # software pipeline: ctx(t-1) -> B(t) -> head(t-1); head matmuls
                # land after B(t)'s dense PE work so ctx latency is hidden
                phase_b(0)
                for t in range(1, NT):
                    phase_cd1(t - 1)
                    phase_b(t)
                    phase_cd2(t - 1)
                phase_cd1(NT - 1)
                phase_cd2(NT - 1)

    nc.compile()
    return nc


def _host_prep(inputs):
    """Build per-core input maps (numpy only)."""
    obs = np.ascontiguousarray(inputs["obs"], dtype=np.float32)
    w_own = np.asarray(inputs["w_own"], np.float32)
    w_int = np.asarray(inputs["w_int"], np.float32)
    wq = np.asarray(inputs["wq"], np.float32)
    wk = np.asarray(inputs["wk"], np.float32)
    wv = np.asarray(inputs["wv"], np.float32)
    v_att = np.asarray(inputs["v_att"], np.float32)
    w_attn = np.asarray(inputs["w_attn"], np.float32)
    w_ownp = np.asarray(inputs["w_ownp"], np.float32)
    w_h1 = np.asarray(inputs["w_h1"], np.float32)
    w_h2 = np.asarray(inputs["w_h2"], np.float32)
    w_out = np.asarray(inputs["w_out"], np.float32)

    def blockdiag(w):  # [H, D, D] -> [126, 126]
        out = np.zeros((TOT, TOT), np.float32)
        for h in range(H):
            out[h * D:(h + 1) * D, h * D:(h + 1) * D] = w[h]
        return out

    wia = np.zeros((126, 18 * 126), np.float32)
    for n in range(18):
        wia[7 * n:7 * n + 7, n * 126:(n + 1) * 126] = w_int
    wib = np.zeros((14, 2 * 126), np.float32)
    for n in range(2):
        wib[7 * n:7 * n + 7, n * 126:(n + 1) * 126] = w_int

    va32 = np.zeros((126, 32), np.float32)
    for h in range(H):
        va32[h * D:(h + 1) * D, h] = v_att[h]

    densel = np.zeros((128, 3), np.float32)
    for j in range(4):
        for h in range(H):
            densel[32 * j + h, h] = 1.0

    ebcsel = np.zeros((128, 4 * 126), np.float32)
    for j in range(4):
        for h in range(H):
            ebcsel[32 * j + h, j * 126 + h * D:(j * 126) + (h + 1) * D] = 1.0

    rbc = np.zeros((3, 126), np.float32)
    for h in range(H):
        rbc[h, h * D:(h + 1) * D] = 1.0

    wh1r = np.ascontiguousarray(
        w_h1.reshape(2, 128, HID).transpose(1, 0, 2).reshape(128, 512))
    wh2r = np.ascontiguousarray(
        w_h2.reshape(2, 128, HID).transpose(1, 0, 2).reshape(128, 512))
    woutr = np.ascontiguousarray(
        w_out.reshape(2, 128, NOUT).transpose(1, 0, 2).reshape(128, 8))

    params = {
        "wia": wia, "wib": wib, "wo": w_own,
        "wqb": blockdiag(wq), "wkb": blockdiag(wk), "wvb": blockdiag(wv),
        "va32": va32.astype(_bf16np), "densel": densel, "ebcsel": ebcsel, "rbc": rbc,
        "wat": w_attn, "wop": w_ownp,
        "wh1r": wh1r, "wh2r": wh2r, "woutr": woutr,
        "bown": np.asarray(inputs["b_own"], np.float32).reshape(126, 1),
        "bint": np.asarray(inputs["b_int"], np.float32).reshape(126, 1),
        "bat": np.asarray(inputs["b_attn"], np.float32).reshape(128, 1),
        "bop": np.asarray(inputs["b_ownp"], np.float32).reshape(128, 1),
        "bh1": np.ascontiguousarray(
            np.asarray(inputs["b_h1"], np.float32).reshape(2, 128).T),
        "bh2": np.ascontiguousarray(
            np.asarray(inputs["b_h2"], np.float32).reshape(2, 128).T),
        "bout": np.asarray(inputs["b_out"], np.float32).reshape(4, 1),
    }

    in_maps = []
    for c in range(N_CORES):
        sl = obs[c * BC:(c + 1) * BC]                       # [BC, 147]
        xo = np.ascontiguousarray(sl[:, :OWN_DIM].T)        # [7, BC]
        intr = sl[:, OWN_DIM:].reshape(BC, N_INTR, INT_DIM)  # [BC, 20, 7]
        intrT = intr.transpose(1, 2, 0)                     # [20, 7, BC]
        xa = np.ascontiguousarray(intrT[:18].reshape(126, BC))
        xb = np.ascontiguousarray(intrT[18:].reshape(14, BC))
        m = {"xo": xo, "xa": xa, "xb": xb}
        m.update(params)
        in_maps.append(m)
    return in_maps


def _get_nc():
    if "nc" not in _BUILT:
        _BUILT["nc"] = _build_nc()
    return _BUILT["nc"]


def run(inputs, trace=False):
    from concourse.bass_utils import run_bass_kernel_spmd
    nc = _get_nc()
    in_maps = _host_prep(inputs)
    res = run_bass_kernel_spmd(nc, in_maps, core_ids=list(range(N_CORES)),
                               trace=trace)
    outs = [res.results[c]["outT"] for c in range(N_CORES)]   # each [4, BC]
    full = np.concatenate(outs, axis=1).T                     # [B, 4]
    return np.ascontiguousarray(full, dtype=np.float32), res


def kernel(**inputs):
    out, _ = run(inputs, trace=False)
    return out


# revision 7
# speedup vs baseline: 1.9609x; 1.0857x over previous
"""Trainium2 Bass kernel for nn_AttentionSACModel (sparse_attention).

Data-parallel across 8 NeuronCores: obs sharded along batch, params replicated.
On-device layout keeps batch on the matmul free dim (activations stored
feature-major / transposed); all host<->device layout changes happen in numpy.
"""
import sys
import os

if "/opt/trn_rl_repo" not in sys.path:
    sys.path.insert(0, "/opt/trn_rl_repo")

import numpy as np
import ml_dtypes
_bf16np = ml_dtypes.bfloat16

OWN_DIM = 7
INT_DIM = 7
N_INTR = 20
H = 3
D = 42
TOT = H * D            # 126
ATTN = 128
HID = 256
NOUT = 4
B = 32768
N_CORES = 8
BC = B // N_CORES      # 4096 rows per core
NB = 512               # batch tile (matmul free dim)
NT = BC // NB          # 8 tiles per core
ALPHA = 0.2            # leaky relu slope

_BUILT = {}


def _build_nc():
    import concourse.bacc as bacc
    import concourse.tile as tile
    from concourse import mybir

    f32 = mybir.dt.float32
    f32r = mybir.dt.float32r
    bf16 = mybir.dt.bfloat16
    AF = mybir.ActivationFunctionType
    ALU = mybir.AluOpType
    AX = mybir.AxisListType

    nc = bacc.Bacc()

    # ---- DRAM I/O ----
    xo_d = nc.dram_tensor("xo", [OWN_DIM, BC], f32r, kind="ExternalInput")
    xa_d = nc.dram_tensor("xa", [126, BC], f32r, kind="ExternalInput")       # interactors 0..17, row 7n+f
    xb_d = nc.dram_tensor("xb", [14, BC], f32r, kind="ExternalInput")        # interactors 18,19
    wia_d = nc.dram_tensor("wia", [126, 18 * 126], f32r, kind="ExternalInput")  # padded int-embed lhsT, n<18
    wib_d = nc.dram_tensor("wib", [14, 2 * 126], f32r, kind="ExternalInput")    # n=18,19
    wo_d = nc.dram_tensor("wo", [7, 126], f32r, kind="ExternalInput")
    wq_d = nc.dram_tensor("wqb", [126, 126], f32r, kind="ExternalInput")
    wk_d = nc.dram_tensor("wkb", [126, 126], f32r, kind="ExternalInput")
    wv_d = nc.dram_tensor("wvb", [126, 126], f32r, kind="ExternalInput")
    va_d = nc.dram_tensor("va32", [126, 32], bf16, kind="ExternalInput")
    ds_d = nc.dram_tensor("densel", [128, 3], f32r, kind="ExternalInput")
    eb_d = nc.dram_tensor("ebcsel", [128, 4 * 126], f32r, kind="ExternalInput")
    rb_d = nc.dram_tensor("rbc", [3, 126], f32r, kind="ExternalInput")
    wat_d = nc.dram_tensor("wat", [126, 128], f32r, kind="ExternalInput")
    wop_d = nc.dram_tensor("wop", [126, 128], f32r, kind="ExternalInput")
    wh1_d = nc.dram_tensor("wh1r", [128, 512], f32r, kind="ExternalInput")   # [p, kc*256+m]
    wh2_d = nc.dram_tensor("wh2r", [128, 512], f32r, kind="ExternalInput")
    wout_d = nc.dram_tensor("woutr", [128, 8], f32r, kind="ExternalInput")   # [p, kc*4+m]
    bown_d = nc.dram_tensor("bown", [126, 1], f32, kind="ExternalInput")
    bint_d = nc.dram_tensor("bint", [126, 1], f32, kind="ExternalInput")
    bat_d = nc.dram_tensor("bat", [128, 1], f32, kind="ExternalInput")
    bop_d = nc.dram_tensor("bop", [128, 1], f32, kind="ExternalInput")
    bh1_d = nc.dram_tensor("bh1", [128, 2], f32, kind="ExternalInput")
    bh2_d = nc.dram_tensor("bh2", [128, 2], f32, kind="ExternalInput")
    bout_d = nc.dram_tensor("bout", [4, 1], f32, kind="ExternalInput")
    out_d = nc.dram_tensor("outT", [NOUT, BC], f32, kind="ExternalOutput")

    with tile.TileContext(nc) as tc:
        with tc.tile_pool(name="const", bufs=1) as cst, \
             tc.tile_pool(name="px", bufs=2) as px, \
             tc.tile_pool(name="pemb", bufs=3) as pemb, \
             tc.tile_pool(name="peng", bufs=3) as peng, \
             tc.tile_pool(name="pE", bufs=10) as pE, \
             tc.tile_pool(name="pv", bufs=2) as pv, \
             tc.tile_pool(name="pp", bufs=2) as pp, \
             tc.tile_pool(name="pn", bufs=6) as pn, \
             tc.tile_pool(name="ph", bufs=2) as ph, \
             tc.tile_pool(name="pz", bufs=2, space="PSUM") as ppz, \
             tc.tile_pool(name="pk", bufs=2, space="PSUM") as ppk, \
             tc.tile_pool(name="sm", bufs=3, space="PSUM") as small, \
             tc.tile_pool(name="pd", bufs=1, space="PSUM") as ppd:

            # ---- load constants ----
            WiA = cst.tile([126, 18 * 126], f32r)
            WiB = cst.tile([14, 2 * 126], f32r)
            Wo = cst.tile([7, 126], f32r)
            Wq = cst.tile([126, 126], f32r)
            Wk = cst.tile([126, 126], f32r)
            Wv = cst.tile([126, 126], f32r)
            Va = cst.tile([126, 32], bf16)
            Ds = cst.tile([128, 3], f32r)
            Eb = cst.tile([128, 4 * 126], f32r)
            Rb = cst.tile([3, 126], f32r)
            Wat = cst.tile([126, 128], f32r)
            Wop = cst.tile([126, 128], f32r)
            WH1 = cst.tile([128, 512], f32r)
            WH2 = cst.tile([128, 512], f32r)
            WOUT = cst.tile([128, 8], f32r)
            Bown = cst.tile([126, 1], f32)
            Bint = cst.tile([126, 1], f32)
            Bat = cst.tile([128, 1], f32)
            Bop = cst.tile([128, 1], f32)
            BH1 = cst.tile([128, 2], f32)
            BH2 = cst.tile([128, 2], f32)
            Bout = cst.tile([4, 1], f32)
            for t_sb, t_dr in [(WiA, wia_d), (WiB, wib_d), (Wo, wo_d), (Wq, wq_d),
                               (Wk, wk_d), (Wv, wv_d), (Va, va_d), (Ds, ds_d),
                               (Eb, eb_d), (Rb, rb_d), (Wat, wat_d), (Wop, wop_d),
                               (WH1, wh1_d), (WH2, wh2_d), (WOUT, wout_d),
                               (Bown, bown_d), (Bint, bint_d), (Bat, bat_d),
                               (Bop, bop_d), (BH1, bh1_d), (BH2, bh2_d),
                               (Bout, bout_d)]:
                nc.sync.dma_start(out=t_sb, in_=t_dr[:, :])

            with nc.allow_low_precision(reason="bf16/f32r intermediates; final accums are f32"):
                state = {}

                def load_x(t):
                    bs = t * NB
                    XO = px.tile([OWN_DIM, NB], f32r, tag="xo", name="XO")
                    XA = px.tile([126, NB], f32r, tag="xa", name="XA")
                    XB = px.tile([14, NB], f32r, tag="xb", name="XB")
                    nc.sync.dma_start(out=XO, in_=xo_d[:, bs:bs + NB])
                    nc.sync.dma_start(out=XA, in_=xa_d[:, bs:bs + NB])
                    nc.sync.dma_start(out=XB, in_=xb_d[:, bs:bs + NB])
                    state[t] = {"X": (XO, XA, XB)}

                def merged(t, tb):
                    """ctx phase of tile t (may be None) interleaved with
                    embed/attention phase of tile tb (may be None)."""
                    st = state.get(t)
                    if st is not None:
                        EGs = st["EGs"]
                        VA = st["VA"]
                        PD = ppd.tile([128, NB], f32, tag="pd", name="PD")
                        for g in range(5):
                            nc.tensor.matmul(PD[0:3, :], Ds, EGs[g],
                                             start=(g == 0), stop=(g == 4))
                        RD = ph.tile([3, NB], f32r, tag="rd", name="RD")
                        nc.vector.reciprocal(RD, PD[0:3, :])
                        PR = small.tile([128, NB], f32, tag="sm", name="PR")
                        nc.tensor.matmul(PR[0:126, :], Rb, RD)
                        TST = pp.tile([126, NB, N_INTR // 2], bf16, tag="tst", name="TST")
                        PNs = []

                    if tb is not None:
                        XO, XA, XB = state[tb]["X"]
                        PO = small.tile([128, NB], f32, tag="sm", name="PO")
                        nc.tensor.matmul(PO[0:126, :], Wo, XO)
                        OWN = ph.tile([126, NB], f32r, tag="own", name="OWN", bufs=4)
                        nc.scalar.activation(OWN, PO[0:126, :], AF.Prelu, bias=Bown, alpha=ALPHA)
                        EGsb = []
                        VAb = pv.tile([126, N_INTR, NB], bf16, tag="va", name="VAb")
                        PS = None
                        ZTs = {}

                        def emit_z(n):
                            PZ = ppz.tile([126, NB], f32, tag="pz", name="PZ")
                            if n < 18:
                                nc.tensor.matmul(PZ, WiA[:, n * 126:(n + 1) * 126], XA)
                            else:
                                nc.tensor.matmul(PZ, WiB[:, (n - 18) * 126:(n - 17) * 126], XB)
                            ZT = pemb.tile([126, NB], f32r, tag="zt", name="ZT")
                            nc.scalar.activation(ZT, PZ, AF.Prelu, bias=Bint, alpha=ALPHA)
                            ZTs[n] = ZT

                        emit_z(0)
                        emit_z(1)

                    for n in range(N_INTR):
                        if tb is not None:
                            ZT = ZTs.pop(n)
                            PK = ppk.tile([126, NB], f32, tag="pk", name="PK")
                            nc.tensor.matmul(PK, Wk, ZT, start=True, stop=False)
                            nc.tensor.matmul(PK, Wq, OWN, start=False, stop=True)
                            EN = peng.tile([126, NB], bf16, tag="en", name="EN")
                            nc.scalar.activation(EN, PK, AF.Tanh)

                            PV = small.tile([128, NB], f32, tag="sm", name="PV")
                            nc.tensor.matmul(PV[0:126, :], Wv, ZT)
                            nc.scalar.activation(VAb[:, n, :], PV[0:126, :], AF.Copy)

                            if n + 2 < N_INTR:
                                emit_z(n + 2)

                            j = n % 4
                            if j == 0:
                                PS = small.tile([128, NB], f32, tag="sm", name="PS")
                            nc.tensor.matmul(PS[32 * j:32 * (j + 1), :], Va, EN,
                                             tile_position=(0, 32 * j))
                            if j == 3:
                                EG = pE.tile([128, NB], f32r, tag="eg", name="EG")
                                nc.scalar.activation(EG, PS, AF.Exp)
                                EGsb.append(EG)

                        if st is not None:
                            g, j = n // 4, n % 4
                            PEb = small.tile([128, NB], f32, tag="sm", name="PEb")
                            nc.tensor.matmul(PEb[0:126, :], Eb[:, j * 126:(j + 1) * 126], EGs[g])
                            PN = pn.tile([126, NB], bf16, tag="pn", name="PN")
                            nc.vector.tensor_tensor(out=PN, in0=PEb[0:126, :],
                                                    in1=VA[:, n, :], op=ALU.mult)
                            PNs.append(PN)
                            if n % 2 == 1:
                                nc.gpsimd.tensor_add(out=TST[:, :, n // 2],
                                                     in0=PNs[n - 1], in1=PNs[n])

                    if st is not None:
                        CTXU = ph.tile([126, NB], f32, tag="ctxu", name="CTXU")
                        nc.vector.tensor_reduce(CTXU, TST[:, :, :], axis=AX.X, op=ALU.add)
                        CTX = ph.tile([126, NB], f32r, tag="ctx", name="CTX")
                        nc.vector.tensor_tensor(out=CTX, in0=CTXU, in1=PR[0:126, :], op=ALU.mult)
                        st["CTX"] = CTX
                    if tb is not None:
                        state[tb].update({"OWN": OWN, "VA": VAb, "EGs": EGsb})

                def head(t):
                    """head MLP + output for tile t"""
                    bs = t * NB
                    OWN = state[t]["OWN"]
                    CTX = state[t]["CTX"]

                    PH1 = small.tile([128, NB], f32, tag="sm", name="PH1")
                    nc.tensor.matmul(PH1, Wat, CTX)
                    ATT = ph.tile([128, NB], f32r, tag="att", name="ATT")
                    nc.scalar.activation(ATT, PH1, AF.Tanh, bias=Bat)

                    PH2 = small.tile([128, NB], f32, tag="sm", name="PH2")
                    nc.tensor.matmul(PH2, Wop, OWN)
                    OWV = ph.tile([128, NB], f32r, tag="owv", name="OWV")
                    nc.scalar.activation(OWV, PH2, AF.Tanh, bias=Bop)

                    H1 = []
                    for mh in range(2):
                        PHh = small.tile([128, NB], f32, tag="sm", name="PHh")
                        nc.tensor.matmul(PHh, WH1[:, mh * 128:(mh + 1) * 128], OWV,
                                         start=True, stop=False)
                        nc.tensor.matmul(PHh, WH1[:, 256 + mh * 128:256 + (mh + 1) * 128], ATT,
                                         start=False, stop=True)
                        H1A = ph.tile([128, NB], f32r, tag=f"h1a{mh}", name="H1A")
                        nc.scalar.activation(H1A, PHh, AF.Prelu, bias=BH1[:, mh:mh + 1], alpha=ALPHA)
                        H1.append(H1A)
                    H2 = []
                    for mh in range(2):
                        PHh2 = small.tile([128, NB], f32, tag="sm", name="PHh2")
                        nc.tensor.matmul(PHh2, WH2[:, mh * 128:(mh + 1) * 128], H1[0],
                                         start=True, stop=False)
                        nc.tensor.matmul(PHh2, WH2[:, 256 + mh * 128:256 + (mh + 1) * 128], H1[1],
                                         start=False, stop=True)
                        H2A = ph.tile([128, NB], f32r, tag=f"h2a{mh}", name="H2A")
                        nc.scalar.activation(H2A, PHh2, AF.Prelu, bias=BH2[:, mh:mh + 1], alpha=ALPHA)
                        H2.append(H2A)

                    PO4 = small.tile([128, NB], f32, tag="sm", name="PO4")
                    nc.tensor.matmul(PO4[0:4, :], WOUT[:, 0:4], H2[0], start=True, stop=False)
                    nc.tensor.matmul(PO4[0:4, :], WOUT[:, 4:8], H2[1], start=False, stop=True)
                    OT = ph.tile([4, NB], f32, tag="ot", name="OT")
                    nc.scalar.activation(OT, PO4[0:4, :], AF.Identity, bias=Bout)
                    nc.sync.dma_start(out=out_d[:, bs:bs + NB], in_=OT)
                    del state[t]

                # 3-deep software pipeline over tiles:
                #   merged(t-1, t): ctx(t-1) + embed/attn(t), then head(t-2)
                load_x(0)
                merged(None, 0)
                for t in range(1, NT):
                    load_x(t)
                    merged(t - 1, t)
                    if t >= 2:
                        head(t - 2)
                merged(NT - 1, None)
                head(NT - 2)
                head(NT - 1)

    nc.compile()
    return nc


def _host_prep(inputs):
    """Build per-core input maps (numpy only)."""
    obs = np.ascontiguousarray(inputs["obs"], dtype=np.float32)
    w_own = np.asarray(inputs["w_own"], np.float32)
    w_int = np.asarray(inputs["w_int"], np.float32)
    wq = np.asarray(inputs["wq"], np.float32)
    wk = np.asarray(inputs["wk"], np.float32)
    wv = np.asarray(inputs["wv"], np.float32)
    v_att = np.asarray(inputs["v_att"], np.float32)
    w_attn = np.asarray(inputs["w_attn"], np.float32)
    w_ownp = np.asarray(inputs["w_ownp"], np.float32)
    w_h1 = np.asarray(inputs["w_h1"], np.float32)
    w_h2 = np.asarray(inputs["w_h2"], np.float32)
    w_out = np.asarray(inputs["w_out"], np.float32)

    def blockdiag(w):  # [H, D, D] -> [126, 126]
        out = np.zeros((TOT, TOT), np.float32)
        for h in range(H):
            out[h * D:(h + 1) * D, h * D:(h + 1) * D] = w[h]
        return out

    wia = np.zeros((126, 18 * 126), np.float32)
    for n in range(18):
        wia[7 * n:7 * n + 7, n * 126:(n + 1) * 126] = w_int
    wib = np.zeros((14, 2 * 126), np.float32)
    for n in range(2):
        wib[7 * n:7 * n + 7, n * 126:(n + 1) * 126] = w_int

    va32 = np.zeros((126, 32), np.float32)
    for h in range(H):
        va32[h * D:(h + 1) * D, h] = v_att[h]

    densel = np.zeros((128, 3), np.float32)
    for j in range(4):
        for h in range(H):
            densel[32 * j + h, h] = 1.0

    ebcsel = np.zeros((128, 4 * 126), np.float32)
    for j in range(4):
        for h in range(H):
            ebcsel[32 * j + h, j * 126 + h * D:(j * 126) + (h + 1) * D] = 1.0

    rbc = np.zeros((3, 126), np.float32)
    for h in range(H):
        rbc[h, h * D:(h + 1) * D] = 1.0

    wh1r = np.ascontiguousarray(
        w_h1.reshape(2, 128, HID).transpose(1, 0, 2).reshape(128, 512))
    wh2r = np.ascontiguousarray(
        w_h2.reshape(2, 128, HID).transpose(1, 0, 2).reshape(128, 512))
    woutr = np.ascontiguousarray(
        w_out.reshape(2, 128, NOUT).transpose(1, 0, 2).reshape(128, 8))

    params = {
        "wia": wia, "wib": wib, "wo": w_own,
        "wqb": blockdiag(wq), "wkb": blockdiag(wk), "wvb": blockdiag(wv),
        "va32": va32.astype(_bf16np), "densel": densel, "ebcsel": ebcsel, "rbc": rbc,
        "wat": w_attn, "wop": w_ownp,
        "wh1r": wh1r, "wh2r": wh2r, "woutr": woutr,
        "bown": np.asarray(inputs["b_own"], np.float32).reshape(126, 1),
        "bint": np.asarray(inputs["b_int"], np.float32).reshape(126, 1),
        "bat": np.asarray(inputs["b_attn"], np.float32).reshape(128, 1),
        "bop": np.asarray(inputs["b_ownp"], np.float32).reshape(128, 1),
        "bh1": np.ascontiguousarray(
            np.asarray(inputs["b_h1"], np.float32).reshape(2, 128).T),
        "bh2": np.ascontiguousarray(
            np.asarray(inputs["b_h2"], np.float32).reshape(2, 128).T),
        "bout": np.asarray(inputs["b_out"], np.float32).reshape(4, 1),
    }

    in_maps = []
    for c in range(N_CORES):
        sl = obs[c * BC:(c + 1) * BC]                       # [BC, 147]
        xo = np.ascontiguousarray(sl[:, :OWN_DIM].T)        # [7, BC]
        intr = sl[:, OWN_DIM:].reshape(BC, N_INTR, INT_DIM)  # [BC, 20, 7]
        intrT = intr.transpose(1, 2, 0)                     # [20, 7, BC]
        xa = np.ascontiguousarray(intrT[:18].reshape(126, BC))
        xb = np.ascontiguousarray(intrT[18:].reshape(14, BC))
        m = {"xo": xo, "xa": xa, "xb": xb}
        m.update(params)
        in_maps.append(m)
    return in_maps


def _get_nc():
    if "nc" not in _BUILT:
        _BUILT["nc"] = _build_nc()
    return _BUILT["nc"]


def run(inputs, trace=False):
    from concourse.bass_utils import run_bass_kernel_spmd
    nc = _get_nc()
    in_maps = _host_prep(inputs)
    res = run_bass_kernel_spmd(nc, in_maps, core_ids=list(range(N_CORES)),
                               trace=trace)
    outs = [res.results[c]["outT"] for c in range(N_CORES)]   # each [4, BC]
    full = np.concatenate(outs, axis=1).T                     # [B, 4]
    return np.ascontiguousarray(full, dtype=np.float32), res


def kernel(**inputs):
    out, _ = run(inputs, trace=False)
    return out
